# revision 20
# baseline (speedup 1.0000x reference)
# GraphSAGE 2-layer GNN on 8 TRN2 NeuronCores.
#
# Strategy (graph/data parallel, per sharding hint):
#   - dst-partition nodes across 8 cores (6250 rows each).
#   - Host: sort edges by (core, window, src), fold 1/(deg+eps) into per-edge
#     weights, build int16 gather-index streams + per-chunk one-hot metadata,
#     padded to a uniform max structure so all cores run one SPMD program.
#   - Device, per layer: bulk dma_gather of edge features (x rows fp32 /
#     h rows bf16) -> DVE builds weighted one-hot [128 edges x 256 rows] ->
#     TensorE segment-sum matmul into PSUM -> weight matmul -> bias(+relu)
#     on ACT -> PE transpose -> SBUF-resident h -> AllGather (bf16) ->
#     layer 2 -> batched log_softmax epilogue.
import sys

sys.path.insert(0, "/opt/trn_rl_repo")

import numpy as np
import ml_dtypes

import concourse.bass as bass
import concourse.bacc as bacc
import concourse.mybir as mybir
import concourse.tile as tile
from concourse.bass_utils import run_bass_kernel_spmd

F32 = mybir.dt.float32
F32R = mybir.dt.float32r
BF16 = mybir.dt.bfloat16
I16 = mybir.dt.int16


class Cfg:
    def __init__(self, N=50000, E=800000, F1=64, F2=128, F3=64, ncores=8,
                 win=256, lo_lim=32768, piece_chunks=64, pair_gap=15):
        self.N, self.E = N, E
        self.F1, self.F2, self.F3 = F1, F2, F3
        self.NC = ncores
        self.WIN = win
        self.LO = lo_lim
        self.PIECE = piece_chunks
        self.G = pair_gap                    # max src-gap for pair descriptors
        self.RPC = N // ncores               # rows per core
        assert self.RPC * ncores == N
        self.NWIN = -(-self.RPC // win)      # windows per core
        self.HALVES = -(-self.RPC // 128)    # 128-row halves per core
        self.RPAD = self.HALVES * 128        # padded rows per core
        self.NPAD = self.RPAD * ncores
        # window-aligned quarters, small last quarter so the final AG (the
        # one L2's first trigger waits on) has minimal data + minimal lag
        qw = sorted(set(min(w, self.NWIN) for w in
                        (0, 9, 17, self.NWIN - 1, self.NWIN)))
        self.QWIN = qw
        self.QN = len(qw) - 1
        self.QROWS = np.array(
            [min(w * win, self.RPAD) for w in qw], np.int64)


def pid_map(cfg, v):
    """Padded node id: rows grouped [quarter][core][row] so each quarter's
    allgather output is one contiguous flat slice of h_full."""
    v = np.asarray(v).astype(np.int64)
    c = v // cfg.RPC
    r = v % cfg.RPC
    q = np.searchsorted(cfg.QROWS, r, side="right") - 1
    qlen = cfg.QROWS[q + 1] - cfg.QROWS[q]
    return cfg.NC * cfg.QROWS[q] + c * qlen + (r - cfg.QROWS[q])


def prep1(cfg, src, dst, deg_w):
    """Layer-1 pair-packed structure.

    Edges are paired within (core, dst-window) when their padded source ids
    differ by g <= G; a pair is served by ONE 256B descriptor into A_g
    (A_g[s] = [x_bf16[s] | x_bf16[s+g]]).  Streams are per (window, half, g)
    with core-uniform slot counts; cores below the max fill slots with
    leftover single edges (degenerate pairs using only the first half).
    """
    NC, WIN, LO, RPC, G = cfg.NC, cfg.WIN, cfg.LO, cfg.RPC, cfg.G
    NWIN = cfg.NWIN
    pid = pid_map(cfg, src)
    core = dst // RPC

    # per (c, w): pair lists per (h, g), single lists per h
    pl_all = [[None] * NWIN for _ in range(NC)]
    sl_all = [[None] * NWIN for _ in range(NC)]
    npair = np.zeros((NC, NWIN, 2, G + 1), np.int64)
    nsing = np.zeros((NC, NWIN, 2), np.int64)
    for c in range(NC):
        m = core == c
        spc = pid[m]
        dl = (dst[m] - c * RPC).astype(np.int64)
        wvc = deg_w[dst[m]]
        wic = dl // WIN
        rowc = dl % WIN
        for w in range(NWIN):
            sel = wic == w
            s = spc[sel]
            r = rowc[sel]
            v = wvc[sel]
            o = np.argsort(s, kind="stable")
            s, r, v = s[o], r[o], v[o]
            pl = {}
            sl = ([], [])
            i, n = 0, len(s)
            while i < n:
                if i + 1 < n and s[i + 1] - s[i] <= G:
                    g = int(s[i + 1] - s[i])
                    h = int(s[i] >= LO)
                    pl.setdefault((h, g), []).append(
                        (s[i], r[i], v[i], r[i + 1], v[i + 1]))
                    i += 2
                else:
                    h = int(s[i] >= LO)
                    sl[h].append((s[i], r[i], v[i]))
                    i += 1
            pl_all[c][w] = pl
            sl_all[c][w] = sl
            for (h, g), lst in pl.items():
                npair[c, w, h, g] = len(lst)
            for h in (0, 1):
                nsing[c, w, h] = len(sl[h])

    base = npair.max(axis=0)                       # [NWIN, 2, G+1]
    slack = base[None] - npair
    absorbed = np.minimum(slack.sum(axis=3), nsing)
    leftover = nsing - absorbed
    extra = leftover.max(axis=0)                   # [NWIN, 2]
    nslot = base.copy()
    nslot[:, :, 0] += extra
    nslot16 = ((nslot + 15) // 16) * 16
    nchg_all = -(-nslot16 // 128)
    nchg_all[nslot16 == 0] = 0

    # class tables: per (w, h) list of (g, idx colbase, local chunk base,
    # nchg, nslot16, meta chunkbase)
    classes = [[[] for _ in range(2)] for _ in range(NWIN)]
    nch_wh = np.zeros((NWIN, 2), np.int64)
    icol = 0
    mchunk = 0
    for w in range(NWIN):
        for h in (0, 1):
            c0 = 0
            for g in range(G + 1):
                ns = int(nslot16[w, h, g])
                if ns == 0:
                    continue
                nchg = int(nchg_all[w, h, g])
                classes[w][h].append((g, icol, c0, nchg, ns, mchunk))
                icol += ns // 16
                c0 += nchg
                mchunk += nchg
            nch_wh[w, h] = c0
    totch1 = mchunk
    icols1 = icol
    nchmax = nch_wh.max(axis=0)                    # per half

    per_core = []
    for c in range(NC):
        idx1 = np.zeros(icols1 * 16, np.int16)
        dstA = np.full((totch1 * 128,), WIN, np.float32)
        wgtA = np.zeros((totch1 * 128,), np.float32)
        dstB = np.full((totch1 * 128,), WIN, np.float32)
        wgtB = np.zeros((totch1 * 128,), np.float32)
        for w in range(NWIN):
            pl = pl_all[c][w]
            sq = {0: list(sl_all[c][w][0]), 1: list(sl_all[c][w][1])}
            for h in (0, 1):
                for (g, cb, c0, nchg, ns, mb) in classes[w][h]:
                    pairs = pl.get((h, g), [])
                    k = len(pairs)
                    cap = ns
                    take = min(cap - k, len(sq[h]))
                    sing = [sq[h].pop() for _ in range(take)] if take > 0 \
                        else []
                    iv = np.zeros(ns, np.int16)
                    da = np.full(ns, WIN, np.float32)
                    wa = np.zeros(ns, np.float32)
                    db = np.full(ns, WIN, np.float32)
                    wb = np.zeros(ns, np.float32)
                    for j, (sp1, rA, vA, rB, vB) in enumerate(pairs):
                        iv[j] = sp1 - h * LO
                        da[j] = rA
                        wa[j] = vA
                        db[j] = rB
                        wb[j] = vB
                    for j, (sp1, rA, vA) in enumerate(sing):
                        iv[k + j] = sp1 - h * LO
                        da[k + j] = rA
                        wa[k + j] = vA
                    idx1[cb * 16:cb * 16 + ns] = iv
                    b = mb * 128
                    dstA[b:b + ns] = da
                    wgtA[b:b + ns] = wa
                    dstB[b:b + ns] = db
                    wgtB[b:b + ns] = wb
            assert not sq[0] and not sq[1], "singles left unplaced"
        idx16 = np.tile(idx1.reshape(-1, 16).T, (8, 1)).copy()
        per_core.append(dict(
            idx1=idx16,
            dstA=dstA.reshape(-1, 128).T.copy(),
            wgtA=wgtA.reshape(-1, 128).T.copy(),
            dstB=dstB.reshape(-1, 128).T.copy(),
            wgtB=wgtB.reshape(-1, 128).T.copy(),
        ))

    struct = dict(classes=classes, nch_wh=nch_wh, nchmax=nchmax,
                  totch1=totch1, icols1=icols1)
    return struct, per_core


def prep(cfg, src, dst, deg_w):
    """Build per-core gather/one-hot metadata with a core-uniform structure.

    Returns (struct, per_core) where struct has the shared max-shape info the
    program builder needs, and per_core the numpy arrays for in_maps.
    """
    NC, WIN, LO, RPC, RPAD = cfg.NC, cfg.WIN, cfg.LO, cfg.RPC, cfg.RPAD
    pid = pid_map(cfg, src)
    core = dst // RPC

    per_core_ed = []
    nlo = np.zeros((NC, cfg.NWIN), np.int64)
    nhi = np.zeros((NC, cfg.NWIN), np.int64)
    for c in range(NC):
        m = core == c
        sp = pid[m]
        dl = (dst[m] - c * RPC).astype(np.int64)
        wv = deg_w[dst[m]]
        wi = dl // WIN
        row = dl % WIN
        hi = (sp >= LO).astype(np.int64)
        order = np.lexsort((sp, hi, wi))
        sp, wv, wi, row, hi = sp[order], wv[order], wi[order], row[order], hi[order]
        per_core_ed.append((sp, wv, wi, row, hi))
        for w in range(cfg.NWIN):
            sel = wi == w
            nlo[c, w] = int((hi[sel] == 0).sum())
            nhi[c, w] = int(hi[sel].sum())

    # uniform chunk structure: per window, max #chunks across cores
    ch_lo = (-(-nlo.max(axis=0) // 128)).astype(np.int64)
    ch_hi = (-(-nhi.max(axis=0) // 128)).astype(np.int64)
    ch_lo = np.maximum(ch_lo, 1)
    ch_hi = np.maximum(ch_hi, 1)
    tot_lo, tot_hi = int(ch_lo.sum()), int(ch_hi.sum())
    totch = tot_lo + tot_hi

    # pieces: consecutive windows with total chunks <= PIECE
    pieces = []
    w0 = 0
    acc = 0
    for w in range(cfg.NWIN):
        cw = int(ch_lo[w] + ch_hi[w])
        assert cw <= cfg.PIECE, "single window exceeds piece budget"
        if acc + cw > cfg.PIECE:
            pieces.append((w0, w))
            w0, acc = w, 0
        acc += cw
    pieces.append((w0, cfg.NWIN))

    LB = np.concatenate([[0], np.cumsum(ch_lo)])   # lo-chunk base per window
    HB = np.concatenate([[0], np.cumsum(ch_hi)])
    MB = np.concatenate([[0], np.cumsum(ch_lo + ch_hi)])  # meta col base

    per_core = []
    for c in range(NC):
        sp, wv, wi, row, hi = per_core_ed[c]
        idx_lo = np.zeros(tot_lo * 128, np.int16)
        idx_hi = np.zeros(tot_hi * 128, np.int16)
        dstrow = np.full((totch * 128,), WIN, np.float32)  # sentinel row
        wgt = np.zeros((totch * 128,), np.float32)
        for w in range(cfg.NWIN):
            sel = wi == w
            sl = sel & (hi == 0)
            sh = sel & (hi == 1)
            klo, khi = int(sl.sum()), int(sh.sum())
            # lo stream
            b = LB[w] * 128
            idx_lo[b:b + klo] = sp[sl].astype(np.int16)
            # hi stream
            b = HB[w] * 128
            idx_hi[b:b + khi] = (sp[sh] - LO).astype(np.int16)
            # meta: lo chunks then hi chunks of this window
            b = MB[w] * 128
            dstrow[b:b + klo] = row[sl].astype(np.float32)
            wgt[b:b + klo] = wv[sl]
            b = (MB[w] + ch_lo[w]) * 128
            dstrow[b:b + khi] = row[sh].astype(np.float32)
            wgt[b:b + khi] = wv[sh]
        # idx arrays -> [16, n/16] interleave (idx i at [i%16, i//16])
        idx = np.concatenate([idx_lo, idx_hi])
        idx16 = np.tile(idx.reshape(-1, 16).T, (8, 1)).copy()
        per_core.append(dict(
            idx=idx16,
            dstrow=dstrow.reshape(-1, 128).T.copy(),
            wgt=wgt.reshape(-1, 128).T.copy(),
        ))

    struct = dict(ch_lo=ch_lo, ch_hi=ch_hi, tot_lo=tot_lo, tot_hi=tot_hi,
                  totch=totch, pieces=pieces, LB=LB, HB=HB, MB=MB)
    return struct, per_core


def build_program(cfg, struct1, struct):
    NC = cfg.NC
    F1, F2, F3, WIN = cfg.F1, cfg.F2, cfg.F3, cfg.WIN
    ch_lo, ch_hi = struct["ch_lo"], struct["ch_hi"]
    LB, HB, MB = struct["LB"], struct["HB"], struct["MB"]
    pieces = struct["pieces"]
    totch = struct["totch"]
    tot_lo = struct["tot_lo"]
    tot_hi = struct["tot_hi"]
    HALVES = cfg.HALVES
    classes1 = struct1["classes"]
    nch_wh = struct1["nch_wh"]
    nchmax1 = struct1["nchmax"]
    totch1 = struct1["totch1"]
    icols1 = struct1["icols1"]

    nc = bacc.Bacc("TRN2", target_bir_lowering=False, debug=False,
                   num_devices=NC)

    ag_d = [nc.dram_tensor(f"ag{g}", [cfg.NPAD, 2 * F1], BF16,
                           kind="ExternalInput") for g in range(cfg.G + 1)]
    W1b = nc.dram_tensor("W1b", [F1, F2], BF16, kind="ExternalInput")
    W2b = nc.dram_tensor("W2b", [F2, F3], BF16, kind="ExternalInput")
    b1d = nc.dram_tensor("b1d", [F2, 1], F32, kind="ExternalInput")
    b2d = nc.dram_tensor("b2d", [F3, 1], F32, kind="ExternalInput")
    iota16 = nc.dram_tensor("iota16", [128, WIN], BF16, kind="ExternalInput")
    ident16 = nc.dram_tensor("ident16", [128, 128], BF16, kind="ExternalInput")
    ident32 = nc.dram_tensor("ident32", [128, 128], F32, kind="ExternalInput")
    idx1_d = nc.dram_tensor("idx1", [128, icols1], I16, kind="ExternalInput")
    dstA_d = nc.dram_tensor("dstA", [128, totch1], F32, kind="ExternalInput")
    wgtA_d = nc.dram_tensor("wgtA", [128, totch1], F32, kind="ExternalInput")
    dstB_d = nc.dram_tensor("dstB", [128, totch1], F32, kind="ExternalInput")
    wgtB_d = nc.dram_tensor("wgtB", [128, totch1], F32, kind="ExternalInput")
    idx_d = nc.dram_tensor("idx", [128, totch * 8], I16, kind="ExternalInput")
    dstrow_d = nc.dram_tensor("dstrow", [128, totch], F32, kind="ExternalInput")
    wgt_d = nc.dram_tensor("wgt", [128, totch], F32, kind="ExternalInput")
    out_d = nc.dram_tensor("out", [128, HALVES, F3], F32, kind="ExternalOutput")

    with tile.TileContext(nc) as tc:
        with (
            tc.tile_pool(name="const", bufs=1) as cpool,
            tc.tile_pool(name="persist", bufs=1) as ppool,
            tc.tile_pool(name="dram", bufs=1, space="DRAM") as dpool,
        ):
            io16 = cpool.tile([128, WIN], BF16)
            nc.sync.dma_start(out=io16[:], in_=iota16[:])
            w1 = cpool.tile([F1, F2], BF16)
            nc.sync.dma_start(out=w1[:], in_=W1b[:])
            w2 = cpool.tile([F2, F3], BF16)
            nc.sync.dma_start(out=w2[:], in_=W2b[:])
            b1 = cpool.tile([F2, 1], F32)
            nc.sync.dma_start(out=b1[:], in_=b1d[:])
            b2 = cpool.tile([F3, 1], F32)
            nc.sync.dma_start(out=b2[:], in_=b2d[:])
            id16 = cpool.tile([128, 128], BF16)
            nc.sync.dma_start(out=id16[:], in_=ident16[:])
            id32 = cpool.tile([128, 128], F32)
            nc.sync.dma_start(out=id32[:], in_=ident32[:])
            idxs = cpool.tile([128, totch * 8], I16)
            idx_csz = -(-totch // 4) * 8
            for ic in range(4):
                a = ic * idx_csz
                b = min((ic + 1) * idx_csz, totch * 8)
                if b > a:
                    nc.sync.dma_start(out=idxs[:, a:b], in_=idx_d[:, a:b])
            dstrow = cpool.tile([128, totch], F32)
            nc.sync.dma_start(out=dstrow[:], in_=dstrow_d[:])
            wgt = cpool.tile([128, totch], F32)
            nc.sync.dma_start(out=wgt[:], in_=wgt_d[:])
            idx1s = cpool.tile([128, icols1], I16)
            i1_csz = -(-icols1 // 4)
            for ic in range(4):
                a = ic * i1_csz
                b = min((ic + 1) * i1_csz, icols1)
                if b > a:
                    nc.sync.dma_start(out=idx1s[:, a:b], in_=idx1_d[:, a:b])
            dstA = cpool.tile([128, totch1], F32)
            nc.sync.dma_start(out=dstA[:], in_=dstA_d[:])
            wgtA = cpool.tile([128, totch1], F32)
            nc.sync.dma_start(out=wgtA[:], in_=wgtA_d[:])
            dstB = cpool.tile([128, totch1], F32)
            nc.sync.dma_start(out=dstB[:], in_=dstB_d[:])
            wgtB = cpool.tile([128, totch1], F32)
            nc.sync.dma_start(out=wgtB[:], in_=wgtB_d[:])

            h_sb = ppool.tile([128, HALVES, F2], BF16)     # layer-1 output rows
            out_sb = ppool.tile([128, HALVES, F3], F32)    # layer-2 logits

            h_shard = dpool.tile([cfg.RPAD, F2], BF16)
            h_full = dpool.tile([cfg.NPAD, F2], BF16)

            def gather(dst_ap, in_ap, col0, n_chunks, elem):
                nc.gpsimd.dma_gather(
                    out_ap=dst_ap,
                    in_ap=in_ap,
                    idxs_ap=idxs[:, col0 * 8:(col0 + n_chunks) * 8],
                    num_idxs=n_chunks * 128,
                    num_idxs_reg=n_chunks * 128,
                    elem_size=elem,
                    single_packet=False,
                )

            def emit_ag(q):
                r0 = int(cfg.QROWS[q])
                r1 = int(cfg.QROWS[q + 1])
                nc.gpsimd.collective_compute(
                    "AllGather",
                    mybir.AluOpType.bypass,
                    replica_groups=[list(range(NC))],
                    ins=[h_shard[r0:r1, :].opt()],
                    outs=[h_full[NC * r0:NC * r1, :].opt()],
                )

            def run_layer1():
                with (
                    tc.tile_pool(name="g1", bufs=2) as gpool,
                    tc.tile_pool(name="oh1", bufs=32) as ohpool,
                    tc.tile_pool(name="ep1", bufs=2) as eppool,
                    tc.tile_pool(name="ps1", bufs=2, space="PSUM") as pspool,
                    tc.tile_pool(name="pt1", bufs=2, space="PSUM") as ptpool,
                ):
                    for w in range(cfg.NWIN):
                        tiles = [None, None]
                        for h in (0, 1):
                            cls = classes1[w][h]
                            if not cls:
                                continue
                            alloc = int(nchmax1[h]) if w < 2 \
                                else int(nch_wh[w, h])
                            t = gpool.tile([128, alloc, 2 * F1], BF16,
                                           tag=f"g1h{h}")
                            if w < 2:
                                # finite contents under every pad slot: a NaN
                                # bit pattern x 0-weight one-hot would poison
                                # the matmul accumulator
                                nc.gpsimd.memset(t[:], 0.0)
                            for (g, cb, c0, nchg, ns, mb) in cls:
                                src = ag_d[g][0:cfg.LO, :] if h == 0 \
                                    else ag_d[g][cfg.LO:cfg.NPAD, :]
                                nc.gpsimd.dma_gather(
                                    out_ap=t[:, c0:c0 + nchg, :],
                                    in_ap=src,
                                    idxs_ap=idx1s[:, cb:cb + ns // 16],
                                    num_idxs=ns,
                                    num_idxs_reg=ns,
                                    elem_size=2 * F1,
                                    single_packet=False,
                                )
                            tiles[h] = t
                        acc = pspool.tile([F1, WIN], F32, tag="acc")
                        nmm = 2 * int(nch_wh[w, 0] + nch_wh[w, 1])
                        k = 0
                        for h in (0, 1):
                            for (g, cb, c0, nchg, ns, mb) in classes1[w][h]:
                                for ck in range(nchg):
                                    mc = mb + ck
                                    ohA = ohpool.tile([128, WIN], BF16,
                                                      tag="oh")
                                    nc.vector.tensor_scalar(
                                        out=ohA[:], in0=io16[:],
                                        scalar1=dstA[:, mc:mc + 1],
                                        scalar2=wgtA[:, mc:mc + 1],
                                        op0=mybir.AluOpType.is_equal,
                                        op1=mybir.AluOpType.mult)
                                    nc.tensor.matmul(
                                        out=acc[:],
                                        lhsT=tiles[h][:, c0 + ck, 0:F1],
                                        rhs=ohA[:],
                                        start=(k == 0), stop=(k == nmm - 1))
                                    k += 1
                                    ohB = ohpool.tile([128, WIN], BF16,
                                                      tag="oh")
                                    nc.vector.tensor_scalar(
                                        out=ohB[:], in0=io16[:],
                                        scalar1=dstB[:, mc:mc + 1],
                                        scalar2=wgtB[:, mc:mc + 1],
                                        op0=mybir.AluOpType.is_equal,
                                        op1=mybir.AluOpType.mult)
                                    nc.tensor.matmul(
                                        out=acc[:],
                                        lhsT=tiles[h][:, c0 + ck, F1:2 * F1],
                                        rhs=ohB[:],
                                        start=(k == 0), stop=(k == nmm - 1))
                                    k += 1
                        # window epilogue
                        mbf = eppool.tile([F1, WIN], BF16, tag="mbf")
                        nc.scalar.activation(
                            out=mbf[:], in_=acc[:],
                            func=mybir.ActivationFunctionType.Identity)
                        z = ptpool.tile([F2, WIN], F32, tag="z")
                        nc.tensor.matmul(out=z[:], lhsT=w1[:],
                                         rhs=mbf[:], start=True,
                                         stop=True)
                        ht = eppool.tile([F2, WIN], BF16, tag="ht")
                        nc.scalar.activation(
                            out=ht[:], in_=z[:],
                            func=mybir.ActivationFunctionType.Relu,
                            bias=b1[:, 0:1])
                        for hf in range(WIN // 128):
                            hh = w * (WIN // 128) + hf
                            if hh >= HALVES:
                                continue
                            tp = ptpool.tile([128, 128], BF16, tag="tp")
                            nc.tensor.transpose(
                                out=tp[:],
                                in_=ht[:, hf * 128:(hf + 1) * 128],
                                identity=id16[:])
                            nc.scalar.activation(
                                out=h_sb[:, hh, :], in_=tp[:],
                                func=mybir.ActivationFunctionType.Identity)
                        # ship this window's h rows to DRAM now so the
                        # allgather input is ready as soon as L1 ends
                        hh0 = w * (WIN // 128)
                        hh1 = min(hh0 + WIN // 128, HALVES)
                        if hh1 > hh0:
                            nc.sync.dma_start(
                                out=h_shard[:].rearrange(
                                    "(hh p) f -> p hh f", p=128)[:, hh0:hh1, :],
                                in_=h_sb[:, hh0:hh1, :])
                        # fire quarter AllGathers mid-stream, LAG windows
                        # after the quarter's last window was issued, so
                        # the Pool engine never stalls waiting for compute
                        for q in range(cfg.QN):
                            if w + 1 == min(cfg.QWIN[q + 1] + 2,
                                            cfg.NWIN) and \
                                    cfg.QWIN[q + 1] + 2 <= cfg.NWIN:
                                emit_ag(q)

            def run_layer(layer):
                assert layer == 2
                elem, gdt = F2, BF16
                src_lo = h_full[0:cfg.LO, :]
                src_hi = h_full[cfg.LO:cfg.NPAD, :]
                with (
                    tc.tile_pool(name=f"g{layer}", bufs=2) as gpool,
                    tc.tile_pool(name=f"oh{layer}", bufs=32) as ohpool,
                    tc.tile_pool(name=f"ep{layer}", bufs=2) as eppool,
                    tc.tile_pool(name=f"ps{layer}", bufs=2, space="PSUM") as pspool,
                    tc.tile_pool(name=f"pt{layer}", bufs=2, space="PSUM") as ptpool,
                ):
                    piece_of = {}
                    g_lo_t = {}
                    g_hi_t = {}
                    for pi, (w0, w1_) in enumerate(pieces):
                        for w in range(w0, w1_):
                            piece_of[w] = pi

                    def need_piece(pi):
                        if pi in g_lo_t:
                            return
                        w0, w1_ = pieces[pi]
                        ncl = int(LB[w1_] - LB[w0])
                        nch = int(HB[w1_] - HB[w0])
                        g_lo = gpool.tile([128, ncl, elem], gdt, tag="glo")
                        g_hi = gpool.tile([128, nch, elem], gdt, tag="ghi")
                        gather(g_lo[:, :, :], src_lo, int(LB[w0]), ncl, elem)
                        gather(g_hi[:, :, :], src_hi, int(tot_lo + HB[w0]),
                               nch, elem)
                        g_lo_t[pi] = g_lo
                        g_hi_t[pi] = g_hi

                    def glo_chunk(w, gc):
                        pi = piece_of[w]
                        need_piece(pi)
                        return g_lo_t[pi][:, gc - int(LB[pieces[pi][0]]), :]

                    def ghi_chunk(w, gc):
                        pi = piece_of[w]
                        need_piece(pi)
                        return g_hi_t[pi][:, gc - int(HB[pieces[pi][0]]), :]

                    for w in range(cfg.NWIN):
                        nl, nh = int(ch_lo[w]), int(ch_hi[w])
                        acc = pspool.tile([F2, WIN], F32, tag="acc")
                        tot = nl + nh
                        for k in range(tot):
                            if k < nl:
                                g = glo_chunk(w, int(LB[w]) + k)
                            else:
                                g = ghi_chunk(w, int(HB[w]) + k - nl)
                            mc = int(MB[w]) + k
                            oh = ohpool.tile([128, WIN], BF16, tag="oh")
                            nc.vector.tensor_scalar(
                                out=oh[:], in0=io16[:],
                                scalar1=dstrow[:, mc:mc + 1],
                                scalar2=wgt[:, mc:mc + 1],
                                op0=mybir.AluOpType.is_equal,
                                op1=mybir.AluOpType.mult)
                            nc.tensor.matmul(
                                out=acc[:], lhsT=g, rhs=oh[:],
                                start=(k == 0), stop=(k == tot - 1))
                        # window epilogue
                        if True:
                            mbf = eppool.tile([F2, WIN], BF16, tag="mbf")
                            nc.scalar.activation(
                                out=mbf[:], in_=acc[:],
                                func=mybir.ActivationFunctionType.Identity)
                            z = ptpool.tile([F3, WIN], F32, tag="z")
                            nc.tensor.matmul(out=z[:], lhsT=w2[:],
                                             rhs=mbf[:], start=True,
                                             stop=True)
                            o2 = eppool.tile([128, WIN], F32, tag="ht")
                            nc.scalar.activation(
                                out=o2[0:F3, :], in_=z[:],
                                func=mybir.ActivationFunctionType.Identity,
                                bias=b2[:, 0:1])
                            for hf in range(WIN // 128):
                                hh = w * (WIN // 128) + hf
                                if hh >= HALVES:
                                    continue
                                tp = ptpool.tile([128, 128], F32, tag="tp")
                                nc.tensor.transpose(
                                    out=tp[:],
                                    in_=o2[:, hf * 128:(hf + 1) * 128],
                                    identity=id32[:])
                                nc.scalar.activation(
                                    out=out_sb[:, hh, :], in_=tp[:, 0:F3],
                                    func=mybir.ActivationFunctionType.Identity)
                            # quarter boundary: log_softmax + ship output rows
                            for q in range(cfg.QN):
                                q0 = int(cfg.QROWS[q]) // 128
                                q1 = int(cfg.QROWS[q + 1]) // 128
                                if w == (q1 - 1) // (WIN // 128):
                                    nh = q1 - q0
                                    nhx = (int(cfg.QROWS[-1]) -
                                           int(cfg.QROWS[-2])) // 128
                                    nhx = max(nhx, nh)
                                    sl = out_sb[:, q0:q1, :]
                                    mx = eppool.tile([128, nhx, 1], F32,
                                                     tag="smx")
                                    nc.vector.reduce_max(
                                        out=mx[:, 0:nh, :], in_=sl,
                                        axis=mybir.AxisListType.X)
                                    nc.vector.tensor_tensor(
                                        out=sl, in0=sl,
                                        in1=mx[:, 0:nh, :].to_broadcast(
                                            [128, nh, F3]),
                                        op=mybir.AluOpType.subtract)
                                    ex = eppool.tile([128, nhx, F3], F32,
                                                     tag="sex")
                                    nc.scalar.activation(
                                        out=ex[:, 0:nh, :], in_=sl,
                                        func=mybir.ActivationFunctionType.Exp)
                                    sm = eppool.tile([128, nhx, 1], F32,
                                                     tag="ssm")
                                    nc.vector.reduce_sum(
                                        out=sm[:, 0:nh, :], in_=ex[:, 0:nh, :],
                                        axis=mybir.AxisListType.X)
                                    ls = eppool.tile([128, nhx, 1], F32,
                                                     tag="sls")
                                    nc.scalar.activation(
                                        out=ls[:, 0:nh, :], in_=sm[:, 0:nh, :],
                                        func=mybir.ActivationFunctionType.Ln)
                                    nc.vector.tensor_tensor(
                                        out=sl, in0=sl,
                                        in1=ls[:, 0:nh, :].to_broadcast(
                                            [128, nh, F3]),
                                        op=mybir.AluOpType.subtract)
                                    nc.sync.dma_start(
                                        out=out_d[:, q0:q1, :], in_=sl)

            import os
            stage = os.environ.get("K_STAGE", "full")
            run_layer1()
            if stage != "l1":
                # quarters whose (end + LAG) passed NWIN fire here, after the
                # last L1 window was issued
                for q in range(cfg.QN):
                    if cfg.QWIN[q + 1] + 2 > cfg.NWIN:
                        emit_ag(q)
            if stage == "full":
                run_layer(2)

    nc.compile()
    return nc


_CACHE = {}


def _get_program(cfg, x, src, dst, W1, b1, W2, b2):
    deg = np.bincount(dst, minlength=cfg.N).astype(np.float64)
    deg_w = (1.0 / (deg + 1e-6)).astype(np.float32)
    struct1, per_core1 = prep1(cfg, src, dst, deg_w)
    struct, per_core = prep(cfg, src, dst, deg_w)

    # bf16 x in padded-id order, plus one pair-layout per gap value g:
    # ag[s] = [x[s] | x[s+g]] so one 256B descriptor serves two edges
    xp = np.zeros((cfg.NPAD + cfg.G + 1, cfg.F1), ml_dtypes.bfloat16)
    xp[pid_map(cfg, np.arange(cfg.N))] = x.astype(ml_dtypes.bfloat16)
    ags = {}
    for g in range(cfg.G + 1):
        ags[f"ag{g}"] = np.concatenate(
            [xp[:cfg.NPAD], xp[g:cfg.NPAD + g]], axis=1)

    iota = np.arange(cfg.WIN, dtype=np.float32)
    shared = dict(
        W1b=W1.astype(ml_dtypes.bfloat16),
        W2b=W2.astype(ml_dtypes.bfloat16),
        b1d=b1.reshape(-1, 1).astype(np.float32),
        b2d=b2.reshape(-1, 1).astype(np.float32),
        iota16=np.tile(iota, (128, 1)).astype(ml_dtypes.bfloat16),
        ident16=np.eye(128, dtype=ml_dtypes.bfloat16),
        ident32=np.eye(128, dtype=np.float32),
        **ags,
    )
    in_maps = []
    for c in range(cfg.NC):
        m = dict(shared)
        m["idx"] = per_core[c]["idx"]
        m["dstrow"] = per_core[c]["dstrow"]
        m["wgt"] = per_core[c]["wgt"]
        m["idx1"] = per_core1[c]["idx1"]
        m["dstA"] = per_core1[c]["dstA"]
        m["wgtA"] = per_core1[c]["wgtA"]
        m["dstB"] = per_core1[c]["dstB"]
        m["wgtB"] = per_core1[c]["wgtB"]
        in_maps.append(m)

    key = (cfg.N, cfg.E, struct["totch"], tuple(struct["ch_lo"]),
           tuple(struct["ch_hi"]), struct1["totch1"], struct1["icols1"],
           tuple(tuple(tuple(cl) for cl in classes_h)
                 for ww in struct1["classes"] for classes_h in ww))
    if key not in _CACHE:
        _CACHE[key] = build_program(cfg, struct1, struct)
    return _CACHE[key], in_maps


def run(cfg, x, src, dst, W1, b1, W2, b2, trace=False, trace_kwargs=None):
    nc, in_maps = _get_program(cfg, x, src, dst, W1, b1, W2, b2)
    res = run_bass_kernel_spmd(nc, in_maps, core_ids=list(range(cfg.NC)),
                               trace=trace, **(trace_kwargs or {}))
    out = np.empty((cfg.N, cfg.F3), np.float32)
    for c in range(cfg.NC):
        o = np.asarray(res.results[c]["out"])  # [128, HALVES, F3]
        o = o.transpose(1, 0, 2).reshape(cfg.RPAD, cfg.F3)
        out[c * cfg.RPC:(c + 1) * cfg.RPC] = o[:cfg.RPC]
    return out, res


def kernel(x, src, dst, W1, b1, W2, b2):
    cfg = Cfg()
    out, _ = run(cfg, np.asarray(x, np.float32), np.asarray(src),
                 np.asarray(dst), np.asarray(W1, np.float32),
                 np.asarray(b1, np.float32), np.asarray(W2, np.float32),
                 np.asarray(b2, np.float32))
    return out



# revision 25
# speedup vs baseline: 1.1899x; 1.1899x over previous
# GraphSAGE 2-layer GNN on 8 TRN2 NeuronCores.
#
# Strategy (graph/data parallel, per sharding hint):
#   - dst-partition nodes across 8 cores (6250 rows each).
#   - Host: sort edges by (core, window, src), fold 1/(deg+eps) into per-edge
#     weights, build int16 gather-index streams + per-chunk one-hot metadata,
#     padded to a uniform max structure so all cores run one SPMD program.
#   - Device, per layer: bulk dma_gather of edge features (x rows fp32 /
#     h rows bf16) -> DVE builds weighted one-hot [128 edges x 256 rows] ->
#     TensorE segment-sum matmul into PSUM -> weight matmul -> bias(+relu)
#     on ACT -> PE transpose -> SBUF-resident h -> AllGather (bf16) ->
#     layer 2 -> batched log_softmax epilogue.
import sys

sys.path.insert(0, "/opt/trn_rl_repo")

import numpy as np
import ml_dtypes

import concourse.bass as bass
import concourse.bacc as bacc
import concourse.mybir as mybir
import concourse.tile as tile
from concourse.bass_utils import run_bass_kernel_spmd

F32 = mybir.dt.float32
F32R = mybir.dt.float32r
BF16 = mybir.dt.bfloat16
I16 = mybir.dt.int16


class Cfg:
    def __init__(self, N=50000, E=800000, F1=64, F2=128, F3=64, ncores=8,
                 win=256, lo_lim=32768, piece_chunks=64, pair_gap=15):
        self.N, self.E = N, E
        self.F1, self.F2, self.F3 = F1, F2, F3
        self.NC = ncores
        self.WIN = win
        self.LO = lo_lim
        self.PIECE = piece_chunks
        self.G = pair_gap                    # max src-gap for pair descriptors
        self.RPC = N // ncores               # rows per core
        assert self.RPC * ncores == N
        self.NWIN = -(-self.RPC // win)      # windows per core
        self.HALVES = -(-self.RPC // 128)    # 128-row halves per core
        self.RPAD = self.HALVES * 128        # padded rows per core
        self.NPAD = self.RPAD * ncores
        # window-aligned quarters, small last quarter so the final AG (the
        # one L2's first trigger waits on) has minimal data + minimal lag
        qw = sorted(set(min(w, self.NWIN) for w in
                        (0, 9, 17, self.NWIN - 1, self.NWIN)))
        self.QWIN = qw
        self.QN = len(qw) - 1
        self.QROWS = np.array(
            [min(w * win, self.RPAD) for w in qw], np.int64)


def pid_map(cfg, v):
    """Padded node id: rows grouped [quarter][core][row] so each quarter's
    allgather output is one contiguous flat slice of h_full."""
    v = np.asarray(v).astype(np.int64)
    c = v // cfg.RPC
    r = v % cfg.RPC
    q = np.searchsorted(cfg.QROWS, r, side="right") - 1
    qlen = cfg.QROWS[q + 1] - cfg.QROWS[q]
    return cfg.NC * cfg.QROWS[q] + c * qlen + (r - cfg.QROWS[q])


def prep1(cfg, src, dst, deg_w):
    """Layer-1 pair-packed structure.

    Edges are paired within (core, dst-window) when their padded source ids
    differ by g <= G; a pair is served by ONE 256B descriptor into A_g
    (A_g[s] = [x_bf16[s] | x_bf16[s+g]]).  Streams are per (window, half, g)
    with core-uniform slot counts; cores below the max fill slots with
    leftover single edges (degenerate pairs using only the first half).
    """
    NC, WIN, LO, RPC, G = cfg.NC, cfg.WIN, cfg.LO, cfg.RPC, cfg.G
    NWIN = cfg.NWIN
    pid = pid_map(cfg, src)
    core = dst // RPC

    # per (c, w): pair lists per (h, g), single lists per h
    pl_all = [[None] * NWIN for _ in range(NC)]
    sl_all = [[None] * NWIN for _ in range(NC)]
    npair = np.zeros((NC, NWIN, 2, G + 1), np.int64)
    nsing = np.zeros((NC, NWIN, 2), np.int64)
    for c in range(NC):
        m = core == c
        spc = pid[m]
        dl = (dst[m] - c * RPC).astype(np.int64)
        wvc = deg_w[dst[m]]
        wic = dl // WIN
        rowc = dl % WIN
        for w in range(NWIN):
            sel = wic == w
            s = spc[sel]
            r = rowc[sel]
            v = wvc[sel]
            o = np.argsort(s, kind="stable")
            s, r, v = s[o], r[o], v[o]
            pl = {}
            sl = ([], [])
            i, n = 0, len(s)
            while i < n:
                if i + 1 < n and s[i + 1] - s[i] <= G:
                    g = int(s[i + 1] - s[i])
                    h = int(s[i] >= LO)
                    pl.setdefault((h, g), []).append(
                        (s[i], r[i], v[i], r[i + 1], v[i + 1]))
                    i += 2
                else:
                    h = int(s[i] >= LO)
                    sl[h].append((s[i], r[i], v[i]))
                    i += 1
            pl_all[c][w] = pl
            sl_all[c][w] = sl
            for (h, g), lst in pl.items():
                npair[c, w, h, g] = len(lst)
            for h in (0, 1):
                nsing[c, w, h] = len(sl[h])

    base = npair.max(axis=0)                       # [NWIN, 2, G+1]
    slack = base[None] - npair
    absorbed = np.minimum(slack.sum(axis=3), nsing)
    leftover = nsing - absorbed
    extra = leftover.max(axis=0)                   # [NWIN, 2]
    nslot = base.copy()
    nslot[:, :, 0] += extra
    nslot16 = ((nslot + 15) // 16) * 16
    nchg_all = -(-nslot16 // 128)
    nchg_all[nslot16 == 0] = 0

    # class tables: per (w, h) list of (g, idx colbase, local chunk base,
    # nchg, nslot16, meta chunkbase)
    classes = [[[] for _ in range(2)] for _ in range(NWIN)]
    nch_wh = np.zeros((NWIN, 2), np.int64)
    icol = 0
    mchunk = 0
    for w in range(NWIN):
        for h in (0, 1):
            c0 = 0
            for g in range(G + 1):
                ns = int(nslot16[w, h, g])
                if ns == 0:
                    continue
                nchg = int(nchg_all[w, h, g])
                classes[w][h].append((g, icol, c0, nchg, ns, mchunk))
                icol += ns // 16
                c0 += nchg
                mchunk += nchg
            nch_wh[w, h] = c0
    totch1 = mchunk
    icols1 = icol
    nchmax = nch_wh.max(axis=0)                    # per half

    per_core = []
    for c in range(NC):
        idx1 = np.zeros(icols1 * 16, np.int16)
        dstA = np.full((totch1 * 128,), WIN, np.float32)
        wgtA = np.zeros((totch1 * 128,), np.float32)
        dstB = np.full((totch1 * 128,), WIN, np.float32)
        wgtB = np.zeros((totch1 * 128,), np.float32)
        for w in range(NWIN):
            pl = pl_all[c][w]
            sq = {0: list(sl_all[c][w][0]), 1: list(sl_all[c][w][1])}
            for h in (0, 1):
                for (g, cb, c0, nchg, ns, mb) in classes[w][h]:
                    pairs = pl.get((h, g), [])
                    k = len(pairs)
                    cap = ns
                    take = min(cap - k, len(sq[h]))
                    sing = [sq[h].pop() for _ in range(take)] if take > 0 \
                        else []
                    iv = np.zeros(ns, np.int16)
                    da = np.full(ns, WIN, np.float32)
                    wa = np.zeros(ns, np.float32)
                    db = np.full(ns, WIN, np.float32)
                    wb = np.zeros(ns, np.float32)
                    for j, (sp1, rA, vA, rB, vB) in enumerate(pairs):
                        iv[j] = sp1 - h * LO
                        da[j] = rA
                        wa[j] = vA
                        db[j] = rB
                        wb[j] = vB
                    for j, (sp1, rA, vA) in enumerate(sing):
                        iv[k + j] = sp1 - h * LO
                        da[k + j] = rA
                        wa[k + j] = vA
                    idx1[cb * 16:cb * 16 + ns] = iv
                    b = mb * 128
                    dstA[b:b + ns] = da
                    wgtA[b:b + ns] = wa
                    dstB[b:b + ns] = db
                    wgtB[b:b + ns] = wb
            assert not sq[0] and not sq[1], "singles left unplaced"
        idx16 = np.tile(idx1.reshape(-1, 16).T, (8, 1)).copy()
        per_core.append(dict(
            idx1=idx16,
            dstA=dstA.reshape(-1, 128).T.copy(),
            wgtA=wgtA.reshape(-1, 128).T.copy(),
            dstB=dstB.reshape(-1, 128).T.copy(),
            wgtB=wgtB.reshape(-1, 128).T.copy(),
        ))

    struct = dict(classes=classes, nch_wh=nch_wh, nchmax=nchmax,
                  totch1=totch1, icols1=icols1)
    return struct, per_core


def prep(cfg, src, dst, deg_w):
    """Build per-core gather/one-hot metadata with a core-uniform structure.

    Returns (struct, per_core) where struct has the shared max-shape info the
    program builder needs, and per_core the numpy arrays for in_maps.
    """
    NC, WIN, LO, RPC, RPAD = cfg.NC, cfg.WIN, cfg.LO, cfg.RPC, cfg.RPAD
    pid = pid_map(cfg, src)
    core = dst // RPC

    per_core_ed = []
    nlo = np.zeros((NC, cfg.NWIN), np.int64)
    nhi = np.zeros((NC, cfg.NWIN), np.int64)
    for c in range(NC):
        m = core == c
        sp = pid[m]
        dl = (dst[m] - c * RPC).astype(np.int64)
        wv = deg_w[dst[m]]
        wi = dl // WIN
        row = dl % WIN
        hi = (sp >= LO).astype(np.int64)
        order = np.lexsort((sp, hi, wi))
        sp, wv, wi, row, hi = sp[order], wv[order], wi[order], row[order], hi[order]
        per_core_ed.append((sp, wv, wi, row, hi))
        for w in range(cfg.NWIN):
            sel = wi == w
            nlo[c, w] = int((hi[sel] == 0).sum())
            nhi[c, w] = int(hi[sel].sum())

    # uniform chunk structure: per window, max #chunks across cores
    ch_lo = (-(-nlo.max(axis=0) // 128)).astype(np.int64)
    ch_hi = (-(-nhi.max(axis=0) // 128)).astype(np.int64)
    ch_lo = np.maximum(ch_lo, 1)
    ch_hi = np.maximum(ch_hi, 1)
    tot_lo, tot_hi = int(ch_lo.sum()), int(ch_hi.sum())
    totch = tot_lo + tot_hi

    # pieces: consecutive windows with total chunks <= PIECE
    pieces = []
    w0 = 0
    acc = 0
    for w in range(cfg.NWIN):
        cw = int(ch_lo[w] + ch_hi[w])
        assert cw <= cfg.PIECE, "single window exceeds piece budget"
        if acc + cw > cfg.PIECE:
            pieces.append((w0, w))
            w0, acc = w, 0
        acc += cw
    pieces.append((w0, cfg.NWIN))

    LB = np.concatenate([[0], np.cumsum(ch_lo)])   # lo-chunk base per window
    HB = np.concatenate([[0], np.cumsum(ch_hi)])
    MB = np.concatenate([[0], np.cumsum(ch_lo + ch_hi)])  # meta col base

    per_core = []
    for c in range(NC):
        sp, wv, wi, row, hi = per_core_ed[c]
        idx_lo = np.zeros(tot_lo * 128, np.int16)
        idx_hi = np.zeros(tot_hi * 128, np.int16)
        dstrow = np.full((totch * 128,), WIN, np.float32)  # sentinel row
        wgt = np.zeros((totch * 128,), np.float32)
        for w in range(cfg.NWIN):
            sel = wi == w
            sl = sel & (hi == 0)
            sh = sel & (hi == 1)
            klo, khi = int(sl.sum()), int(sh.sum())
            # lo stream
            b = LB[w] * 128
            idx_lo[b:b + klo] = sp[sl].astype(np.int16)
            # hi stream
            b = HB[w] * 128
            idx_hi[b:b + khi] = (sp[sh] - LO).astype(np.int16)
            # meta: lo chunks then hi chunks of this window
            b = MB[w] * 128
            dstrow[b:b + klo] = row[sl].astype(np.float32)
            wgt[b:b + klo] = wv[sl]
            b = (MB[w] + ch_lo[w]) * 128
            dstrow[b:b + khi] = row[sh].astype(np.float32)
            wgt[b:b + khi] = wv[sh]
        # idx arrays -> [16, n/16] interleave (idx i at [i%16, i//16])
        idx = np.concatenate([idx_lo, idx_hi])
        idx16 = np.tile(idx.reshape(-1, 16).T, (8, 1)).copy()
        per_core.append(dict(
            idx=idx16,
            dstrow=dstrow.reshape(-1, 128).T.copy(),
            wgt=wgt.reshape(-1, 128).T.copy(),
        ))

    struct = dict(ch_lo=ch_lo, ch_hi=ch_hi, tot_lo=tot_lo, tot_hi=tot_hi,
                  totch=totch, pieces=pieces, LB=LB, HB=HB, MB=MB)
    return struct, per_core


def build_program(cfg, struct1, struct):
    NC = cfg.NC
    F1, F2, F3, WIN = cfg.F1, cfg.F2, cfg.F3, cfg.WIN
    ch_lo, ch_hi = struct["ch_lo"], struct["ch_hi"]
    LB, HB, MB = struct["LB"], struct["HB"], struct["MB"]
    pieces = struct["pieces"]
    totch = struct["totch"]
    tot_lo = struct["tot_lo"]
    tot_hi = struct["tot_hi"]
    HALVES = cfg.HALVES
    classes1 = struct1["classes"]
    nch_wh = struct1["nch_wh"]
    nchmax1 = struct1["nchmax"]
    totch1 = struct1["totch1"]
    icols1 = struct1["icols1"]

    nc = bacc.Bacc("TRN2", target_bir_lowering=False, debug=False,
                   num_devices=NC)

    ag_d = [nc.dram_tensor(f"ag{g}", [cfg.NPAD, 2 * F1], BF16,
                           kind="ExternalInput") for g in range(cfg.G + 1)]
    W1b = nc.dram_tensor("W1b", [F1, F2], BF16, kind="ExternalInput")
    W2b = nc.dram_tensor("W2b", [F2, F3], BF16, kind="ExternalInput")
    b1d = nc.dram_tensor("b1d", [F2, 1], F32, kind="ExternalInput")
    b2d = nc.dram_tensor("b2d", [F3, 1], F32, kind="ExternalInput")
    iota16 = nc.dram_tensor("iota16", [128, WIN], BF16, kind="ExternalInput")
    ident16 = nc.dram_tensor("ident16", [128, 128], BF16, kind="ExternalInput")
    ident32 = nc.dram_tensor("ident32", [128, 128], F32, kind="ExternalInput")
    idx1_d = nc.dram_tensor("idx1", [128, icols1], I16, kind="ExternalInput")
    dstA_d = nc.dram_tensor("dstA", [128, totch1], F32, kind="ExternalInput")
    wgtA_d = nc.dram_tensor("wgtA", [128, totch1], F32, kind="ExternalInput")
    dstB_d = nc.dram_tensor("dstB", [128, totch1], F32, kind="ExternalInput")
    wgtB_d = nc.dram_tensor("wgtB", [128, totch1], F32, kind="ExternalInput")
    idx_d = nc.dram_tensor("idx", [128, totch * 8], I16, kind="ExternalInput")
    dstrow_d = nc.dram_tensor("dstrow", [128, totch], F32, kind="ExternalInput")
    wgt_d = nc.dram_tensor("wgt", [128, totch], F32, kind="ExternalInput")
    out_d = nc.dram_tensor("out", [128, HALVES, F3], F32, kind="ExternalOutput")

    with tile.TileContext(nc) as tc:
        with (
            tc.tile_pool(name="const", bufs=1) as cpool,
            tc.tile_pool(name="persist", bufs=1) as ppool,
            tc.tile_pool(name="dram", bufs=1, space="DRAM") as dpool,
        ):
            io16 = cpool.tile([128, WIN], BF16)
            nc.sync.dma_start(out=io16[:], in_=iota16[:])
            w1 = cpool.tile([F1, F2], BF16)
            nc.sync.dma_start(out=w1[:], in_=W1b[:])
            w2 = cpool.tile([F2, F3], BF16)
            nc.sync.dma_start(out=w2[:], in_=W2b[:])
            b1 = cpool.tile([F2, 1], F32)
            nc.sync.dma_start(out=b1[:], in_=b1d[:])
            b2 = cpool.tile([F3, 1], F32)
            nc.sync.dma_start(out=b2[:], in_=b2d[:])
            id16 = cpool.tile([128, 128], BF16)
            nc.sync.dma_start(out=id16[:], in_=ident16[:])
            id32 = cpool.tile([128, 128], F32)
            nc.sync.dma_start(out=id32[:], in_=ident32[:])
            idxs = cpool.tile([128, totch * 8], I16)
            idx_csz = -(-totch // 4) * 8
            for ic in range(4):
                a = ic * idx_csz
                b = min((ic + 1) * idx_csz, totch * 8)
                if b > a:
                    nc.sync.dma_start(out=idxs[:, a:b], in_=idx_d[:, a:b])
            dstrow = cpool.tile([128, totch], F32)
            nc.sync.dma_start(out=dstrow[:], in_=dstrow_d[:])
            wgt = cpool.tile([128, totch], F32)
            nc.sync.dma_start(out=wgt[:], in_=wgt_d[:])
            idx1s = cpool.tile([128, icols1], I16)
            i1_csz = -(-icols1 // 4)
            for ic in range(4):
                a = ic * i1_csz
                b = min((ic + 1) * i1_csz, icols1)
                if b > a:
                    nc.sync.dma_start(out=idx1s[:, a:b], in_=idx1_d[:, a:b])
            dstA = cpool.tile([128, totch1], F32)
            nc.sync.dma_start(out=dstA[:], in_=dstA_d[:])
            wgtA = cpool.tile([128, totch1], F32)
            nc.sync.dma_start(out=wgtA[:], in_=wgtA_d[:])
            dstB = cpool.tile([128, totch1], F32)
            nc.sync.dma_start(out=dstB[:], in_=dstB_d[:])
            wgtB = cpool.tile([128, totch1], F32)
            nc.sync.dma_start(out=wgtB[:], in_=wgtB_d[:])

            h_sb = ppool.tile([128, HALVES, F2], BF16)     # layer-1 output rows
            out_sb = ppool.tile([128, HALVES, F3], F32)    # layer-2 logits

            h_shard = dpool.tile([cfg.RPAD, F2], BF16)
            h_full = dpool.tile([cfg.NPAD, F2], BF16)

            def gather(dst_ap, in_ap, col0, n_chunks, elem):
                nc.gpsimd.dma_gather(
                    out_ap=dst_ap,
                    in_ap=in_ap,
                    idxs_ap=idxs[:, col0 * 8:(col0 + n_chunks) * 8],
                    num_idxs=n_chunks * 128,
                    num_idxs_reg=n_chunks * 128,
                    elem_size=elem,
                    single_packet=False,
                )

            def emit_ag(q):
                r0 = int(cfg.QROWS[q])
                r1 = int(cfg.QROWS[q + 1])
                nc.gpsimd.collective_compute(
                    "AllGather",
                    mybir.AluOpType.bypass,
                    replica_groups=[list(range(NC))],
                    ins=[h_shard[r0:r1, :].opt()],
                    outs=[h_full[NC * r0:NC * r1, :].opt()],
                )

            def run_layer1():
                with (
                    tc.tile_pool(name="g1", bufs=2) as gpool,
                    tc.tile_pool(name="oh1", bufs=32) as ohpool,
                    tc.tile_pool(name="ep1", bufs=2) as eppool,
                    tc.tile_pool(name="ps1", bufs=2, space="PSUM") as pspool,
                    tc.tile_pool(name="pt1", bufs=2, space="PSUM") as ptpool,
                ):
                    for w in range(cfg.NWIN):
                        tiles = [None, None]
                        for h in (0, 1):
                            cls = classes1[w][h]
                            if not cls:
                                continue
                            alloc = int(nchmax1[h]) if w < 2 \
                                else int(nch_wh[w, h])
                            t = gpool.tile([128, alloc, 2 * F1], BF16,
                                           tag=f"g1h{h}")
                            if w < 2:
                                # finite contents under every pad slot: a NaN
                                # bit pattern x 0-weight one-hot would poison
                                # the matmul accumulator
                                nc.gpsimd.memset(t[:], 0.0)
                            for (g, cb, c0, nchg, ns, mb) in cls:
                                src = ag_d[g][0:cfg.LO, :] if h == 0 \
                                    else ag_d[g][cfg.LO:cfg.NPAD, :]
                                nc.gpsimd.dma_gather(
                                    out_ap=t[:, c0:c0 + nchg, :],
                                    in_ap=src,
                                    idxs_ap=idx1s[:, cb:cb + ns // 16],
                                    num_idxs=ns,
                                    num_idxs_reg=ns,
                                    elem_size=2 * F1,
                                    single_packet=False,
                                )
                            tiles[h] = t
                        acc = pspool.tile([F1, WIN], F32, tag="acc")
                        nmm = 2 * int(nch_wh[w, 0] + nch_wh[w, 1])
                        k = 0
                        for h in (0, 1):
                            for (g, cb, c0, nchg, ns, mb) in classes1[w][h]:
                                for ck in range(nchg):
                                    mc = mb + ck
                                    ohA = ohpool.tile([128, WIN], BF16,
                                                      tag="oh")
                                    nc.vector.tensor_scalar(
                                        out=ohA[:], in0=io16[:],
                                        scalar1=dstA[:, mc:mc + 1],
                                        scalar2=wgtA[:, mc:mc + 1],
                                        op0=mybir.AluOpType.is_equal,
                                        op1=mybir.AluOpType.mult)
                                    nc.tensor.matmul(
                                        out=acc[:],
                                        lhsT=tiles[h][:, c0 + ck, 0:F1],
                                        rhs=ohA[:],
                                        start=(k == 0), stop=(k == nmm - 1))
                                    k += 1
                                    ohB = ohpool.tile([128, WIN], BF16,
                                                      tag="oh")
                                    nc.vector.tensor_scalar(
                                        out=ohB[:], in0=io16[:],
                                        scalar1=dstB[:, mc:mc + 1],
                                        scalar2=wgtB[:, mc:mc + 1],
                                        op0=mybir.AluOpType.is_equal,
                                        op1=mybir.AluOpType.mult)
                                    nc.tensor.matmul(
                                        out=acc[:],
                                        lhsT=tiles[h][:, c0 + ck, F1:2 * F1],
                                        rhs=ohB[:],
                                        start=(k == 0), stop=(k == nmm - 1))
                                    k += 1
                        # window epilogue
                        mbf = eppool.tile([F1, WIN], BF16, tag="mbf")
                        nc.scalar.activation(
                            out=mbf[:], in_=acc[:],
                            func=mybir.ActivationFunctionType.Identity)
                        z = ptpool.tile([F2, WIN], F32, tag="z")
                        nc.tensor.matmul(out=z[:], lhsT=w1[:],
                                         rhs=mbf[:], start=True,
                                         stop=True)
                        ht = eppool.tile([F2, WIN], BF16, tag="ht")
                        nc.scalar.activation(
                            out=ht[:], in_=z[:],
                            func=mybir.ActivationFunctionType.Relu,
                            bias=b1[:, 0:1])
                        for hf in range(WIN // 128):
                            hh = w * (WIN // 128) + hf
                            if hh >= HALVES:
                                continue
                            tp = ptpool.tile([128, 128], BF16, tag="tp")
                            nc.tensor.transpose(
                                out=tp[:],
                                in_=ht[:, hf * 128:(hf + 1) * 128],
                                identity=id16[:])
                            nc.scalar.activation(
                                out=h_sb[:, hh, :], in_=tp[:],
                                func=mybir.ActivationFunctionType.Identity)
                        # ship this window's h rows to DRAM now so the
                        # allgather input is ready as soon as L1 ends
                        hh0 = w * (WIN // 128)
                        hh1 = min(hh0 + WIN // 128, HALVES)
                        if hh1 > hh0:
                            nc.sync.dma_start(
                                out=h_shard[:].rearrange(
                                    "(hh p) f -> p hh f", p=128)[:, hh0:hh1, :],
                                in_=h_sb[:, hh0:hh1, :])
                        # fire quarter AllGathers mid-stream, LAG windows
                        # after the quarter's last window was issued, so
                        # the Pool engine never stalls waiting for compute
                        for q in range(cfg.QN):
                            if w + 1 == min(cfg.QWIN[q + 1] + 2,
                                            cfg.NWIN) and \
                                    cfg.QWIN[q + 1] + 2 <= cfg.NWIN:
                                emit_ag(q)

            def run_layer(layer):
                assert layer == 2
                elem, gdt = F2, BF16
                src_lo = h_full[0:cfg.LO, :]
                src_hi = h_full[cfg.LO:cfg.NPAD, :]
                with (
                    tc.tile_pool(name=f"g{layer}", bufs=2) as gpool,
                    tc.tile_pool(name=f"oh{layer}", bufs=32) as ohpool,
                    tc.tile_pool(name=f"ep{layer}", bufs=2) as eppool,
                    tc.tile_pool(name=f"ps{layer}", bufs=2, space="PSUM") as pspool,
                    tc.tile_pool(name=f"pt{layer}", bufs=2, space="PSUM") as ptpool,
                ):
                    piece_of = {}
                    g_lo_t = {}
                    g_hi_t = {}
                    for pi, (w0, w1_) in enumerate(pieces):
                        for w in range(w0, w1_):
                            piece_of[w] = pi

                    def need_piece(pi):
                        if pi in g_lo_t:
                            return
                        w0, w1_ = pieces[pi]
                        ncl = int(LB[w1_] - LB[w0])
                        nch = int(HB[w1_] - HB[w0])
                        g_lo = gpool.tile([128, ncl, elem], gdt, tag="glo")
                        g_hi = gpool.tile([128, nch, elem], gdt, tag="ghi")
                        gather(g_lo[:, :, :], src_lo, int(LB[w0]), ncl, elem)
                        gather(g_hi[:, :, :], src_hi, int(tot_lo + HB[w0]),
                               nch, elem)
                        g_lo_t[pi] = g_lo
                        g_hi_t[pi] = g_hi

                    def glo_chunk(w, gc):
                        pi = piece_of[w]
                        need_piece(pi)
                        return g_lo_t[pi][:, gc - int(LB[pieces[pi][0]]), :]

                    def ghi_chunk(w, gc):
                        pi = piece_of[w]
                        need_piece(pi)
                        return g_hi_t[pi][:, gc - int(HB[pieces[pi][0]]), :]

                    for w in range(cfg.NWIN):
                        nl, nh = int(ch_lo[w]), int(ch_hi[w])
                        acc = pspool.tile([F2, WIN], F32, tag="acc")
                        tot = nl + nh
                        for k in range(tot):
                            if k < nl:
                                g = glo_chunk(w, int(LB[w]) + k)
                            else:
                                g = ghi_chunk(w, int(HB[w]) + k - nl)
                            mc = int(MB[w]) + k
                            oh = ohpool.tile([128, WIN], BF16, tag="oh")
                            nc.vector.tensor_scalar(
                                out=oh[:], in0=io16[:],
                                scalar1=dstrow[:, mc:mc + 1],
                                scalar2=wgt[:, mc:mc + 1],
                                op0=mybir.AluOpType.is_equal,
                                op1=mybir.AluOpType.mult)
                            nc.tensor.matmul(
                                out=acc[:], lhsT=g, rhs=oh[:],
                                start=(k == 0), stop=(k == tot - 1))
                        # window epilogue
                        if True:
                            mbf = eppool.tile([F2, WIN], BF16, tag="mbf")
                            nc.scalar.activation(
                                out=mbf[:], in_=acc[:],
                                func=mybir.ActivationFunctionType.Identity)
                            z = ptpool.tile([F3, WIN], F32, tag="z")
                            nc.tensor.matmul(out=z[:], lhsT=w2[:],
                                             rhs=mbf[:], start=True,
                                             stop=True)
                            o2 = eppool.tile([128, WIN], F32, tag="ht")
                            nc.scalar.activation(
                                out=o2[0:F3, :], in_=z[:],
                                func=mybir.ActivationFunctionType.Identity,
                                bias=b2[:, 0:1])
                            for hf in range(WIN // 128):
                                hh = w * (WIN // 128) + hf
                                if hh >= HALVES:
                                    continue
                                tp = ptpool.tile([128, 128], F32, tag="tp")
                                nc.tensor.transpose(
                                    out=tp[:],
                                    in_=o2[:, hf * 128:(hf + 1) * 128],
                                    identity=id32[:])
                                nc.scalar.activation(
                                    out=out_sb[:, hh, :], in_=tp[:, 0:F3],
                                    func=mybir.ActivationFunctionType.Identity)
                            # quarter boundary: log_softmax + ship output rows
                            for q in range(cfg.QN):
                                q0 = int(cfg.QROWS[q]) // 128
                                q1 = int(cfg.QROWS[q + 1]) // 128
                                if w == (q1 - 1) // (WIN // 128):
                                    nh = q1 - q0
                                    nhx = (int(cfg.QROWS[-1]) -
                                           int(cfg.QROWS[-2])) // 128
                                    nhx = max(nhx, nh)
                                    sl = out_sb[:, q0:q1, :]
                                    mx = eppool.tile([128, nhx, 1], F32,
                                                     tag="smx")
                                    nc.vector.reduce_max(
                                        out=mx[:, 0:nh, :], in_=sl,
                                        axis=mybir.AxisListType.X)
                                    nc.vector.tensor_tensor(
                                        out=sl, in0=sl,
                                        in1=mx[:, 0:nh, :].to_broadcast(
                                            [128, nh, F3]),
                                        op=mybir.AluOpType.subtract)
                                    ex = eppool.tile([128, nhx, F3], F32,
                                                     tag="sex")
                                    nc.scalar.activation(
                                        out=ex[:, 0:nh, :], in_=sl,
                                        func=mybir.ActivationFunctionType.Exp)
                                    sm = eppool.tile([128, nhx, 1], F32,
                                                     tag="ssm")
                                    nc.vector.reduce_sum(
                                        out=sm[:, 0:nh, :], in_=ex[:, 0:nh, :],
                                        axis=mybir.AxisListType.X)
                                    ls = eppool.tile([128, nhx, 1], F32,
                                                     tag="sls")
                                    nc.scalar.activation(
                                        out=ls[:, 0:nh, :], in_=sm[:, 0:nh, :],
                                        func=mybir.ActivationFunctionType.Ln)
                                    nc.vector.tensor_tensor(
                                        out=sl, in0=sl,
                                        in1=ls[:, 0:nh, :].to_broadcast(
                                            [128, nh, F3]),
                                        op=mybir.AluOpType.subtract)
                                    nc.sync.dma_start(
                                        out=out_d[:, q0:q1, :], in_=sl)

            import os
            stage = os.environ.get("K_STAGE", "full")
            run_layer1()
            if stage != "l1":
                # quarters whose (end + LAG) passed NWIN fire here, after the
                # last L1 window was issued
                for q in range(cfg.QN):
                    if cfg.QWIN[q + 1] + 2 > cfg.NWIN:
                        emit_ag(q)
            if stage == "full":
                run_layer(2)

    nc.compile()
    return nc


_CACHE = {}


def _get_program(cfg, x, src, dst, W1, b1, W2, b2):
    deg = np.bincount(dst, minlength=cfg.N).astype(np.float64)
    deg_w = (1.0 / (deg + 1e-6)).astype(np.float32)
    struct1, per_core1 = prep1(cfg, src, dst, deg_w)
    struct, per_core = prep(cfg, src, dst, deg_w)

    # bf16 x in padded-id order, plus one pair-layout per gap value g:
    # ag[s] = [x[s] | x[s+g]] so one 256B descriptor serves two edges
    xp = np.zeros((cfg.NPAD + cfg.G + 1, cfg.F1), ml_dtypes.bfloat16)
    xp[pid_map(cfg, np.arange(cfg.N))] = x.astype(ml_dtypes.bfloat16)
    ags = {}
    for g in range(cfg.G + 1):
        ags[f"ag{g}"] = np.concatenate(
            [xp[:cfg.NPAD], xp[g:cfg.NPAD + g]], axis=1)

    iota = np.arange(cfg.WIN, dtype=np.float32)
    shared = dict(
        W1b=W1.astype(ml_dtypes.bfloat16),
        W2b=W2.astype(ml_dtypes.bfloat16),
        b1d=b1.reshape(-1, 1).astype(np.float32),
        b2d=b2.reshape(-1, 1).astype(np.float32),
        iota16=np.tile(iota, (128, 1)).astype(ml_dtypes.bfloat16),
        ident16=np.eye(128, dtype=ml_dtypes.bfloat16),
        ident32=np.eye(128, dtype=np.float32),
        **ags,
    )
    in_maps = []
    for c in range(cfg.NC):
        m = dict(shared)
        m["idx"] = per_core[c]["idx"]
        m["dstrow"] = per_core[c]["dstrow"]
        m["wgt"] = per_core[c]["wgt"]
        m["idx1"] = per_core1[c]["idx1"]
        m["dstA"] = per_core1[c]["dstA"]
        m["wgtA"] = per_core1[c]["wgtA"]
        m["dstB"] = per_core1[c]["dstB"]
        m["wgtB"] = per_core1[c]["wgtB"]
        in_maps.append(m)

    key = (cfg.N, cfg.E, struct["totch"], tuple(struct["ch_lo"]),
           tuple(struct["ch_hi"]), struct1["totch1"], struct1["icols1"],
           tuple(tuple(tuple(cl) for cl in classes_h)
                 for ww in struct1["classes"] for classes_h in ww))
    if key not in _CACHE:
        _CACHE[key] = build_program(cfg, struct1, struct)
    return _CACHE[key], in_maps


def run(cfg, x, src, dst, W1, b1, W2, b2, trace=False, trace_kwargs=None):
    nc, in_maps = _get_program(cfg, x, src, dst, W1, b1, W2, b2)
    res = run_bass_kernel_spmd(nc, in_maps, core_ids=list(range(cfg.NC)),
                               trace=trace, **(trace_kwargs or {}))
    out = np.empty((cfg.N, cfg.F3), np.float32)
    for c in range(cfg.NC):
        o = np.asarray(res.results[c]["out"])  # [128, HALVES, F3]
        o = o.transpose(1, 0, 2).reshape(cfg.RPAD, cfg.F3)
        out[c * cfg.RPC:(c + 1) * cfg.RPC] = o[:cfg.RPC]
    return out, res


def kernel(x, src, dst, W1, b1, W2, b2):
    cfg = Cfg()
    out, _ = run(cfg, np.asarray(x, np.float32), np.asarray(src),
                 np.asarray(dst), np.asarray(W1, np.float32),
                 np.asarray(b1, np.float32), np.asarray(W2, np.float32),
                 np.asarray(b2, np.float32))
    return out



# revision 26
# speedup vs baseline: 1.2153x; 1.0213x over previous
# GraphSAGE 2-layer GNN on 8 TRN2 NeuronCores.
#
# Strategy (graph/data parallel, per sharding hint):
#   - dst-partition nodes across 8 cores (6250 rows each).
#   - Host: sort edges by (core, window, src), fold 1/(deg+eps) into per-edge
#     weights, build int16 gather-index streams + per-chunk one-hot metadata,
#     padded to a uniform max structure so all cores run one SPMD program.
#   - Device, per layer: bulk dma_gather of edge features (x rows fp32 /
#     h rows bf16) -> DVE builds weighted one-hot [128 edges x 256 rows] ->
#     TensorE segment-sum matmul into PSUM -> weight matmul -> bias(+relu)
#     on ACT -> PE transpose -> SBUF-resident h -> AllGather (bf16) ->
#     layer 2 -> batched log_softmax epilogue.
import sys

sys.path.insert(0, "/opt/trn_rl_repo")

import numpy as np
import ml_dtypes

import concourse.bass as bass
import concourse.bacc as bacc
import concourse.mybir as mybir
import concourse.tile as tile
from concourse.bass_utils import run_bass_kernel_spmd

F32 = mybir.dt.float32
F32R = mybir.dt.float32r
BF16 = mybir.dt.bfloat16
I16 = mybir.dt.int16


class Cfg:
    def __init__(self, N=50000, E=800000, F1=64, F2=128, F3=64, ncores=8,
                 win=256, lo_lim=32768, piece_chunks=70, pair_gap=15):
        self.N, self.E = N, E
        self.F1, self.F2, self.F3 = F1, F2, F3
        self.NC = ncores
        self.WIN = win
        self.LO = lo_lim
        self.PIECE = piece_chunks
        self.G = pair_gap                    # max src-gap for pair descriptors
        self.RPC = N // ncores               # rows per core
        assert self.RPC * ncores == N
        self.NWIN = -(-self.RPC // win)      # windows per core
        self.HALVES = -(-self.RPC // 128)    # 128-row halves per core
        self.RPAD = self.HALVES * 128        # padded rows per core
        self.NPAD = self.RPAD * ncores
        # window-aligned quarters, small last quarter so the final AG (the
        # one L2's first trigger waits on) has minimal data + minimal lag
        qw = sorted(set(min(w, self.NWIN) for w in
                        (0, 9, 17, self.NWIN - 1, self.NWIN)))
        self.QWIN = qw
        self.QN = len(qw) - 1
        self.QROWS = np.array(
            [min(w * win, self.RPAD) for w in qw], np.int64)


def pid_map(cfg, v):
    """Padded node id: rows grouped [quarter][core][row] so each quarter's
    allgather output is one contiguous flat slice of h_full."""
    v = np.asarray(v).astype(np.int64)
    c = v // cfg.RPC
    r = v % cfg.RPC
    q = np.searchsorted(cfg.QROWS, r, side="right") - 1
    qlen = cfg.QROWS[q + 1] - cfg.QROWS[q]
    return cfg.NC * cfg.QROWS[q] + c * qlen + (r - cfg.QROWS[q])


def prep1(cfg, src, dst, deg_w):
    """Layer-1 pair-packed structure.

    Edges are paired within (core, dst-window) when their padded source ids
    differ by g <= G; a pair is served by ONE 256B descriptor into A_g
    (A_g[s] = [x_bf16[s] | x_bf16[s+g]]).  Streams are per (window, half, g)
    with core-uniform slot counts; cores below the max fill slots with
    leftover single edges (degenerate pairs using only the first half).
    """
    NC, WIN, LO, RPC, G = cfg.NC, cfg.WIN, cfg.LO, cfg.RPC, cfg.G
    NWIN = cfg.NWIN
    pid = pid_map(cfg, src)
    core = dst // RPC

    # per (c, w): pair lists per (h, g), single lists per h
    pl_all = [[None] * NWIN for _ in range(NC)]
    sl_all = [[None] * NWIN for _ in range(NC)]
    npair = np.zeros((NC, NWIN, 2, G + 1), np.int64)
    nsing = np.zeros((NC, NWIN, 2), np.int64)
    for c in range(NC):
        m = core == c
        spc = pid[m]
        dl = (dst[m] - c * RPC).astype(np.int64)
        wvc = deg_w[dst[m]]
        wic = dl // WIN
        rowc = dl % WIN
        for w in range(NWIN):
            sel = wic == w
            s = spc[sel]
            r = rowc[sel]
            v = wvc[sel]
            o = np.argsort(s, kind="stable")
            s, r, v = s[o], r[o], v[o]
            pl = {}
            sl = ([], [])
            i, n = 0, len(s)
            while i < n:
                if i + 1 < n and s[i + 1] - s[i] <= G:
                    g = int(s[i + 1] - s[i])
                    h = int(s[i] >= LO)
                    pl.setdefault((h, g), []).append(
                        (s[i], r[i], v[i], r[i + 1], v[i + 1]))
                    i += 2
                else:
                    h = int(s[i] >= LO)
                    sl[h].append((s[i], r[i], v[i]))
                    i += 1
            pl_all[c][w] = pl
            sl_all[c][w] = sl
            for (h, g), lst in pl.items():
                npair[c, w, h, g] = len(lst)
            for h in (0, 1):
                nsing[c, w, h] = len(sl[h])

    base = npair.max(axis=0)                       # [NWIN, 2, G+1]
    slack = base[None] - npair
    absorbed = np.minimum(slack.sum(axis=3), nsing)
    leftover = nsing - absorbed
    extra = leftover.max(axis=0)                   # [NWIN, 2]
    nslot = base.copy()
    nslot[:, :, 0] += extra
    nslot16 = ((nslot + 15) // 16) * 16
    nchg_all = -(-nslot16 // 128)
    nchg_all[nslot16 == 0] = 0

    # class tables: per (w, h) list of (g, idx colbase, local chunk base,
    # nchg, nslot16, meta chunkbase)
    classes = [[[] for _ in range(2)] for _ in range(NWIN)]
    nch_wh = np.zeros((NWIN, 2), np.int64)
    icol = 0
    mchunk = 0
    for w in range(NWIN):
        for h in (0, 1):
            c0 = 0
            for g in range(G + 1):
                ns = int(nslot16[w, h, g])
                if ns == 0:
                    continue
                nchg = int(nchg_all[w, h, g])
                classes[w][h].append((g, icol, c0, nchg, ns, mchunk))
                icol += ns // 16
                c0 += nchg
                mchunk += nchg
            nch_wh[w, h] = c0
    totch1 = mchunk
    icols1 = icol
    nchmax = nch_wh.max(axis=0)                    # per half

    per_core = []
    for c in range(NC):
        idx1 = np.zeros(icols1 * 16, np.int16)
        dstA = np.full((totch1 * 128,), WIN, np.float32)
        wgtA = np.zeros((totch1 * 128,), np.float32)
        dstB = np.full((totch1 * 128,), WIN, np.float32)
        wgtB = np.zeros((totch1 * 128,), np.float32)
        for w in range(NWIN):
            pl = pl_all[c][w]
            sq = {0: list(sl_all[c][w][0]), 1: list(sl_all[c][w][1])}
            for h in (0, 1):
                for (g, cb, c0, nchg, ns, mb) in classes[w][h]:
                    pairs = pl.get((h, g), [])
                    k = len(pairs)
                    cap = ns
                    take = min(cap - k, len(sq[h]))
                    sing = [sq[h].pop() for _ in range(take)] if take > 0 \
                        else []
                    iv = np.zeros(ns, np.int16)
                    da = np.full(ns, WIN, np.float32)
                    wa = np.zeros(ns, np.float32)
                    db = np.full(ns, WIN, np.float32)
                    wb = np.zeros(ns, np.float32)
                    for j, (sp1, rA, vA, rB, vB) in enumerate(pairs):
                        iv[j] = sp1 - h * LO
                        da[j] = rA
                        wa[j] = vA
                        db[j] = rB
                        wb[j] = vB
                    for j, (sp1, rA, vA) in enumerate(sing):
                        iv[k + j] = sp1 - h * LO
                        da[k + j] = rA
                        wa[k + j] = vA
                    idx1[cb * 16:cb * 16 + ns] = iv
                    b = mb * 128
                    dstA[b:b + ns] = da
                    wgtA[b:b + ns] = wa
                    dstB[b:b + ns] = db
                    wgtB[b:b + ns] = wb
            assert not sq[0] and not sq[1], "singles left unplaced"
        idx16 = np.tile(idx1.reshape(-1, 16).T, (8, 1)).copy()
        per_core.append(dict(
            idx1=idx16,
            dstA=dstA.reshape(-1, 128).T.copy(),
            wgtA=wgtA.reshape(-1, 128).T.copy(),
            dstB=dstB.reshape(-1, 128).T.copy(),
            wgtB=wgtB.reshape(-1, 128).T.copy(),
        ))

    struct = dict(classes=classes, nch_wh=nch_wh, nchmax=nchmax,
                  totch1=totch1, icols1=icols1)
    return struct, per_core


def prep(cfg, src, dst, deg_w):
    """Build per-core gather/one-hot metadata with a core-uniform structure.

    Returns (struct, per_core) where struct has the shared max-shape info the
    program builder needs, and per_core the numpy arrays for in_maps.
    """
    NC, WIN, LO, RPC, RPAD = cfg.NC, cfg.WIN, cfg.LO, cfg.RPC, cfg.RPAD
    pid = pid_map(cfg, src)
    core = dst // RPC

    per_core_ed = []
    nlo = np.zeros((NC, cfg.NWIN), np.int64)
    nhi = np.zeros((NC, cfg.NWIN), np.int64)
    for c in range(NC):
        m = core == c
        sp = pid[m]
        dl = (dst[m] - c * RPC).astype(np.int64)
        wv = deg_w[dst[m]]
        wi = dl // WIN
        row = dl % WIN
        hi = (sp >= LO).astype(np.int64)
        order = np.lexsort((sp, hi, wi))
        sp, wv, wi, row, hi = sp[order], wv[order], wi[order], row[order], hi[order]
        per_core_ed.append((sp, wv, wi, row, hi))
        for w in range(cfg.NWIN):
            sel = wi == w
            nlo[c, w] = int((hi[sel] == 0).sum())
            nhi[c, w] = int(hi[sel].sum())

    # uniform chunk structure: per window, max #chunks across cores
    ch_lo = (-(-nlo.max(axis=0) // 128)).astype(np.int64)
    ch_hi = (-(-nhi.max(axis=0) // 128)).astype(np.int64)
    ch_lo = np.maximum(ch_lo, 1)
    ch_hi = np.maximum(ch_hi, 1)
    tot_lo, tot_hi = int(ch_lo.sum()), int(ch_hi.sum())
    totch = tot_lo + tot_hi

    # pieces: consecutive windows with total chunks <= PIECE
    pieces = []
    w0 = 0
    acc = 0
    for w in range(cfg.NWIN):
        cw = int(ch_lo[w] + ch_hi[w])
        assert cw <= cfg.PIECE, "single window exceeds piece budget"
        if acc + cw > cfg.PIECE:
            pieces.append((w0, w))
            w0, acc = w, 0
        acc += cw
    pieces.append((w0, cfg.NWIN))

    LB = np.concatenate([[0], np.cumsum(ch_lo)])   # lo-chunk base per window
    HB = np.concatenate([[0], np.cumsum(ch_hi)])
    MB = np.concatenate([[0], np.cumsum(ch_lo + ch_hi)])  # meta col base

    per_core = []
    for c in range(NC):
        sp, wv, wi, row, hi = per_core_ed[c]
        idx_lo = np.zeros(tot_lo * 128, np.int16)
        idx_hi = np.zeros(tot_hi * 128, np.int16)
        dstrow = np.full((totch * 128,), WIN, np.float32)  # sentinel row
        wgt = np.zeros((totch * 128,), np.float32)
        for w in range(cfg.NWIN):
            sel = wi == w
            sl = sel & (hi == 0)
            sh = sel & (hi == 1)
            klo, khi = int(sl.sum()), int(sh.sum())
            # lo stream
            b = LB[w] * 128
            idx_lo[b:b + klo] = sp[sl].astype(np.int16)
            # hi stream
            b = HB[w] * 128
            idx_hi[b:b + khi] = (sp[sh] - LO).astype(np.int16)
            # meta: lo chunks then hi chunks of this window
            b = MB[w] * 128
            dstrow[b:b + klo] = row[sl].astype(np.float32)
            wgt[b:b + klo] = wv[sl]
            b = (MB[w] + ch_lo[w]) * 128
            dstrow[b:b + khi] = row[sh].astype(np.float32)
            wgt[b:b + khi] = wv[sh]
        # idx arrays -> [16, n/16] interleave (idx i at [i%16, i//16])
        idx = np.concatenate([idx_lo, idx_hi])
        idx16 = np.tile(idx.reshape(-1, 16).T, (8, 1)).copy()
        per_core.append(dict(
            idx=idx16,
            dstrow=dstrow.reshape(-1, 128).T.copy(),
            wgt=wgt.reshape(-1, 128).T.copy(),
        ))

    struct = dict(ch_lo=ch_lo, ch_hi=ch_hi, tot_lo=tot_lo, tot_hi=tot_hi,
                  totch=totch, pieces=pieces, LB=LB, HB=HB, MB=MB)
    return struct, per_core


def build_program(cfg, struct1, struct):
    NC = cfg.NC
    F1, F2, F3, WIN = cfg.F1, cfg.F2, cfg.F3, cfg.WIN
    ch_lo, ch_hi = struct["ch_lo"], struct["ch_hi"]
    LB, HB, MB = struct["LB"], struct["HB"], struct["MB"]
    pieces = struct["pieces"]
    totch = struct["totch"]
    tot_lo = struct["tot_lo"]
    tot_hi = struct["tot_hi"]
    HALVES = cfg.HALVES
    classes1 = struct1["classes"]
    nch_wh = struct1["nch_wh"]
    nchmax1 = struct1["nchmax"]
    totch1 = struct1["totch1"]
    icols1 = struct1["icols1"]

    nc = bacc.Bacc("TRN2", target_bir_lowering=False, debug=False,
                   num_devices=NC)

    ag_d = [nc.dram_tensor(f"ag{g}", [cfg.NPAD, 2 * F1], BF16,
                           kind="ExternalInput") for g in range(cfg.G + 1)]
    W1b = nc.dram_tensor("W1b", [F1, F2], BF16, kind="ExternalInput")
    W2b = nc.dram_tensor("W2b", [F2, F3], BF16, kind="ExternalInput")
    b1d = nc.dram_tensor("b1d", [F2, 1], F32, kind="ExternalInput")
    b2d = nc.dram_tensor("b2d", [F3, 1], F32, kind="ExternalInput")
    iota16 = nc.dram_tensor("iota16", [128, WIN], BF16, kind="ExternalInput")
    ident16 = nc.dram_tensor("ident16", [128, 128], BF16, kind="ExternalInput")
    ident32 = nc.dram_tensor("ident32", [128, 128], F32, kind="ExternalInput")
    idx1_d = nc.dram_tensor("idx1", [128, icols1], I16, kind="ExternalInput")
    dstA_d = nc.dram_tensor("dstA", [128, totch1], F32, kind="ExternalInput")
    wgtA_d = nc.dram_tensor("wgtA", [128, totch1], F32, kind="ExternalInput")
    dstB_d = nc.dram_tensor("dstB", [128, totch1], F32, kind="ExternalInput")
    wgtB_d = nc.dram_tensor("wgtB", [128, totch1], F32, kind="ExternalInput")
    idx_d = nc.dram_tensor("idx", [128, totch * 8], I16, kind="ExternalInput")
    dstrow_d = nc.dram_tensor("dstrow", [128, totch], F32, kind="ExternalInput")
    wgt_d = nc.dram_tensor("wgt", [128, totch], F32, kind="ExternalInput")
    out_d = nc.dram_tensor("out", [128, HALVES, F3], F32, kind="ExternalOutput")

    with tile.TileContext(nc) as tc:
        with (
            tc.tile_pool(name="const", bufs=1) as cpool,
            tc.tile_pool(name="persist", bufs=1) as ppool,
            tc.tile_pool(name="dram", bufs=1, space="DRAM") as dpool,
        ):
            io16 = cpool.tile([128, WIN], BF16)
            nc.sync.dma_start(out=io16[:], in_=iota16[:])
            w1 = cpool.tile([F1, F2], BF16)
            nc.sync.dma_start(out=w1[:], in_=W1b[:])
            w2 = cpool.tile([F2, F3], BF16)
            nc.sync.dma_start(out=w2[:], in_=W2b[:])
            b1 = cpool.tile([F2, 1], F32)
            nc.sync.dma_start(out=b1[:], in_=b1d[:])
            b2 = cpool.tile([F3, 1], F32)
            nc.sync.dma_start(out=b2[:], in_=b2d[:])
            id16 = cpool.tile([128, 128], BF16)
            nc.sync.dma_start(out=id16[:], in_=ident16[:])
            id32 = cpool.tile([128, 128], F32)
            nc.sync.dma_start(out=id32[:], in_=ident32[:])
            idxs = cpool.tile([128, totch * 8], I16)
            idx_csz = -(-totch // 4) * 8
            for ic in range(4):
                a = ic * idx_csz
                b = min((ic + 1) * idx_csz, totch * 8)
                if b > a:
                    nc.sync.dma_start(out=idxs[:, a:b], in_=idx_d[:, a:b])
            dstrow = cpool.tile([128, totch], F32)
            nc.sync.dma_start(out=dstrow[:], in_=dstrow_d[:])
            wgt = cpool.tile([128, totch], F32)
            nc.sync.dma_start(out=wgt[:], in_=wgt_d[:])
            idx1s = cpool.tile([128, icols1], I16)
            i1_csz = -(-icols1 // 4)
            for ic in range(4):
                a = ic * i1_csz
                b = min((ic + 1) * i1_csz, icols1)
                if b > a:
                    nc.sync.dma_start(out=idx1s[:, a:b], in_=idx1_d[:, a:b])
            dstA = cpool.tile([128, totch1], F32)
            nc.sync.dma_start(out=dstA[:], in_=dstA_d[:])
            wgtA = cpool.tile([128, totch1], F32)
            nc.sync.dma_start(out=wgtA[:], in_=wgtA_d[:])
            dstB = cpool.tile([128, totch1], F32)
            nc.sync.dma_start(out=dstB[:], in_=dstB_d[:])
            wgtB = cpool.tile([128, totch1], F32)
            nc.sync.dma_start(out=wgtB[:], in_=wgtB_d[:])

            h_sb = ppool.tile([128, HALVES, F2], BF16)     # layer-1 output rows
            out_sb = ppool.tile([128, HALVES, F3], F32)    # layer-2 logits

            h_shard = dpool.tile([cfg.RPAD, F2], BF16)
            h_full = dpool.tile([cfg.NPAD, F2], BF16)

            def gather(dst_ap, in_ap, col0, n_chunks, elem):
                nc.gpsimd.dma_gather(
                    out_ap=dst_ap,
                    in_ap=in_ap,
                    idxs_ap=idxs[:, col0 * 8:(col0 + n_chunks) * 8],
                    num_idxs=n_chunks * 128,
                    num_idxs_reg=n_chunks * 128,
                    elem_size=elem,
                    single_packet=False,
                )

            def emit_ag(q):
                r0 = int(cfg.QROWS[q])
                r1 = int(cfg.QROWS[q + 1])
                nc.gpsimd.collective_compute(
                    "AllGather",
                    mybir.AluOpType.bypass,
                    replica_groups=[list(range(NC))],
                    ins=[h_shard[r0:r1, :].opt()],
                    outs=[h_full[NC * r0:NC * r1, :].opt()],
                )

            def run_layer1():
                with (
                    tc.tile_pool(name="g1", bufs=2) as gpool,
                    tc.tile_pool(name="oh1", bufs=32) as ohpool,
                    tc.tile_pool(name="ep1", bufs=2) as eppool,
                    tc.tile_pool(name="ps1", bufs=2, space="PSUM") as pspool,
                    tc.tile_pool(name="pt1", bufs=2, space="PSUM") as ptpool,
                ):
                    for w in range(cfg.NWIN):
                        tiles = [None, None]
                        for h in (0, 1):
                            cls = classes1[w][h]
                            if not cls:
                                continue
                            alloc = int(nchmax1[h]) if w < 2 \
                                else int(nch_wh[w, h])
                            t = gpool.tile([128, alloc, 2 * F1], BF16,
                                           tag=f"g1h{h}")
                            if w < 2:
                                # finite contents under every pad slot: a NaN
                                # bit pattern x 0-weight one-hot would poison
                                # the matmul accumulator
                                nc.gpsimd.memset(t[:], 0.0)
                            for (g, cb, c0, nchg, ns, mb) in cls:
                                src = ag_d[g][0:cfg.LO, :] if h == 0 \
                                    else ag_d[g][cfg.LO:cfg.NPAD, :]
                                nc.gpsimd.dma_gather(
                                    out_ap=t[:, c0:c0 + nchg, :],
                                    in_ap=src,
                                    idxs_ap=idx1s[:, cb:cb + ns // 16],
                                    num_idxs=ns,
                                    num_idxs_reg=ns,
                                    elem_size=2 * F1,
                                    single_packet=False,
                                )
                            tiles[h] = t
                        acc = pspool.tile([F1, WIN], F32, tag="acc")
                        nmm = 2 * int(nch_wh[w, 0] + nch_wh[w, 1])
                        k = 0
                        for h in (0, 1):
                            for (g, cb, c0, nchg, ns, mb) in classes1[w][h]:
                                for ck in range(nchg):
                                    mc = mb + ck
                                    ohA = ohpool.tile([128, WIN], BF16,
                                                      tag="oh")
                                    nc.vector.tensor_scalar(
                                        out=ohA[:], in0=io16[:],
                                        scalar1=dstA[:, mc:mc + 1],
                                        scalar2=wgtA[:, mc:mc + 1],
                                        op0=mybir.AluOpType.is_equal,
                                        op1=mybir.AluOpType.mult)
                                    nc.tensor.matmul(
                                        out=acc[:],
                                        lhsT=tiles[h][:, c0 + ck, 0:F1],
                                        rhs=ohA[:],
                                        start=(k == 0), stop=(k == nmm - 1))
                                    k += 1
                                    ohB = ohpool.tile([128, WIN], BF16,
                                                      tag="oh")
                                    nc.vector.tensor_scalar(
                                        out=ohB[:], in0=io16[:],
                                        scalar1=dstB[:, mc:mc + 1],
                                        scalar2=wgtB[:, mc:mc + 1],
                                        op0=mybir.AluOpType.is_equal,
                                        op1=mybir.AluOpType.mult)
                                    nc.tensor.matmul(
                                        out=acc[:],
                                        lhsT=tiles[h][:, c0 + ck, F1:2 * F1],
                                        rhs=ohB[:],
                                        start=(k == 0), stop=(k == nmm - 1))
                                    k += 1
                        # window epilogue
                        mbf = eppool.tile([F1, WIN], BF16, tag="mbf")
                        nc.scalar.activation(
                            out=mbf[:], in_=acc[:],
                            func=mybir.ActivationFunctionType.Identity)
                        z = ptpool.tile([F2, WIN], F32, tag="z")
                        nc.tensor.matmul(out=z[:], lhsT=w1[:],
                                         rhs=mbf[:], start=True,
                                         stop=True)
                        ht = eppool.tile([F2, WIN], BF16, tag="ht")
                        nc.scalar.activation(
                            out=ht[:], in_=z[:],
                            func=mybir.ActivationFunctionType.Relu,
                            bias=b1[:, 0:1])
                        for hf in range(WIN // 128):
                            hh = w * (WIN // 128) + hf
                            if hh >= HALVES:
                                continue
                            tp = ptpool.tile([128, 128], BF16, tag="tp")
                            nc.tensor.transpose(
                                out=tp[:],
                                in_=ht[:, hf * 128:(hf + 1) * 128],
                                identity=id16[:])
                            nc.scalar.activation(
                                out=h_sb[:, hh, :], in_=tp[:],
                                func=mybir.ActivationFunctionType.Identity)
                        # ship this window's h rows to DRAM now so the
                        # allgather input is ready as soon as L1 ends
                        hh0 = w * (WIN // 128)
                        hh1 = min(hh0 + WIN // 128, HALVES)
                        if hh1 > hh0:
                            nc.sync.dma_start(
                                out=h_shard[:].rearrange(
                                    "(hh p) f -> p hh f", p=128)[:, hh0:hh1, :],
                                in_=h_sb[:, hh0:hh1, :])
                        # fire quarter AllGathers mid-stream, LAG windows
                        # after the quarter's last window was issued, so
                        # the Pool engine never stalls waiting for compute
                        for q in range(cfg.QN):
                            if w + 1 == min(cfg.QWIN[q + 1] + 2,
                                            cfg.NWIN) and \
                                    cfg.QWIN[q + 1] + 2 <= cfg.NWIN:
                                emit_ag(q)

            def run_layer(layer):
                assert layer == 2
                elem, gdt = F2, BF16
                src_lo = h_full[0:cfg.LO, :]
                src_hi = h_full[cfg.LO:cfg.NPAD, :]
                with (
                    tc.tile_pool(name=f"g{layer}", bufs=2) as gpool,
                    tc.tile_pool(name=f"oh{layer}", bufs=32) as ohpool,
                    tc.tile_pool(name=f"ep{layer}", bufs=2) as eppool,
                    tc.tile_pool(name=f"ps{layer}", bufs=2, space="PSUM") as pspool,
                    tc.tile_pool(name=f"pt{layer}", bufs=2, space="PSUM") as ptpool,
                ):
                    piece_of = {}
                    g_lo_t = {}
                    g_hi_t = {}
                    for pi, (w0, w1_) in enumerate(pieces):
                        for w in range(w0, w1_):
                            piece_of[w] = pi

                    def need_piece(pi):
                        if pi in g_lo_t:
                            return
                        w0, w1_ = pieces[pi]
                        ncl = int(LB[w1_] - LB[w0])
                        nch = int(HB[w1_] - HB[w0])
                        g_lo = gpool.tile([128, ncl, elem], gdt, tag="glo")
                        g_hi = gpool.tile([128, nch, elem], gdt, tag="ghi")
                        gather(g_lo[:, :, :], src_lo, int(LB[w0]), ncl, elem)
                        gather(g_hi[:, :, :], src_hi, int(tot_lo + HB[w0]),
                               nch, elem)
                        g_lo_t[pi] = g_lo
                        g_hi_t[pi] = g_hi

                    def glo_chunk(w, gc):
                        pi = piece_of[w]
                        need_piece(pi)
                        return g_lo_t[pi][:, gc - int(LB[pieces[pi][0]]), :]

                    def ghi_chunk(w, gc):
                        pi = piece_of[w]
                        need_piece(pi)
                        return g_hi_t[pi][:, gc - int(HB[pieces[pi][0]]), :]

                    for w in range(cfg.NWIN):
                        nl, nh = int(ch_lo[w]), int(ch_hi[w])
                        acc = pspool.tile([F2, WIN], F32, tag="acc")
                        tot = nl + nh
                        for k in range(tot):
                            if k < nl:
                                g = glo_chunk(w, int(LB[w]) + k)
                            else:
                                g = ghi_chunk(w, int(HB[w]) + k - nl)
                            mc = int(MB[w]) + k
                            oh = ohpool.tile([128, WIN], BF16, tag="oh")
                            nc.vector.tensor_scalar(
                                out=oh[:], in0=io16[:],
                                scalar1=dstrow[:, mc:mc + 1],
                                scalar2=wgt[:, mc:mc + 1],
                                op0=mybir.AluOpType.is_equal,
                                op1=mybir.AluOpType.mult)
                            nc.tensor.matmul(
                                out=acc[:], lhsT=g, rhs=oh[:],
                                start=(k == 0), stop=(k == tot - 1))
                        # window epilogue
                        if True:
                            mbf = eppool.tile([F2, WIN], BF16, tag="mbf")
                            nc.scalar.activation(
                                out=mbf[:], in_=acc[:],
                                func=mybir.ActivationFunctionType.Identity)
                            z = ptpool.tile([F3, WIN], F32, tag="z")
                            nc.tensor.matmul(out=z[:], lhsT=w2[:],
                                             rhs=mbf[:], start=True,
                                             stop=True)
                            o2 = eppool.tile([128, WIN], F32, tag="ht")
                            nc.scalar.activation(
                                out=o2[0:F3, :], in_=z[:],
                                func=mybir.ActivationFunctionType.Identity,
                                bias=b2[:, 0:1])
                            for hf in range(WIN // 128):
                                hh = w * (WIN // 128) + hf
                                if hh >= HALVES:
                                    continue
                                tp = ptpool.tile([128, 128], F32, tag="tp")
                                nc.tensor.transpose(
                                    out=tp[:],
                                    in_=o2[:, hf * 128:(hf + 1) * 128],
                                    identity=id32[:])
                                nc.scalar.activation(
                                    out=out_sb[:, hh, :], in_=tp[:, 0:F3],
                                    func=mybir.ActivationFunctionType.Identity)
                            # quarter boundary: log_softmax + ship output rows
                            for q in range(cfg.QN):
                                q0 = int(cfg.QROWS[q]) // 128
                                q1 = int(cfg.QROWS[q + 1]) // 128
                                if w == (q1 - 1) // (WIN // 128):
                                    nh = q1 - q0
                                    nhx = (int(cfg.QROWS[-1]) -
                                           int(cfg.QROWS[-2])) // 128
                                    nhx = max(nhx, nh)
                                    sl = out_sb[:, q0:q1, :]
                                    mx = eppool.tile([128, nhx, 1], F32,
                                                     tag="smx")
                                    nc.vector.reduce_max(
                                        out=mx[:, 0:nh, :], in_=sl,
                                        axis=mybir.AxisListType.X)
                                    nc.vector.tensor_tensor(
                                        out=sl, in0=sl,
                                        in1=mx[:, 0:nh, :].to_broadcast(
                                            [128, nh, F3]),
                                        op=mybir.AluOpType.subtract)
                                    ex = eppool.tile([128, nhx, F3], F32,
                                                     tag="sex")
                                    nc.scalar.activation(
                                        out=ex[:, 0:nh, :], in_=sl,
                                        func=mybir.ActivationFunctionType.Exp)
                                    sm = eppool.tile([128, nhx, 1], F32,
                                                     tag="ssm")
                                    nc.vector.reduce_sum(
                                        out=sm[:, 0:nh, :], in_=ex[:, 0:nh, :],
                                        axis=mybir.AxisListType.X)
                                    ls = eppool.tile([128, nhx, 1], F32,
                                                     tag="sls")
                                    nc.scalar.activation(
                                        out=ls[:, 0:nh, :], in_=sm[:, 0:nh, :],
                                        func=mybir.ActivationFunctionType.Ln)
                                    nc.vector.tensor_tensor(
                                        out=sl, in0=sl,
                                        in1=ls[:, 0:nh, :].to_broadcast(
                                            [128, nh, F3]),
                                        op=mybir.AluOpType.subtract)
                                    nc.sync.dma_start(
                                        out=out_d[:, q0:q1, :], in_=sl)

            import os
            stage = os.environ.get("K_STAGE", "full")
            run_layer1()
            if stage != "l1":
                # quarters whose (end + LAG) passed NWIN fire here, after the
                # last L1 window was issued
                for q in range(cfg.QN):
                    if cfg.QWIN[q + 1] + 2 > cfg.NWIN:
                        emit_ag(q)
            if stage == "full":
                run_layer(2)

    nc.compile()
    return nc


_CACHE = {}


def _get_program(cfg, x, src, dst, W1, b1, W2, b2):
    deg = np.bincount(dst, minlength=cfg.N).astype(np.float64)
    deg_w = (1.0 / (deg + 1e-6)).astype(np.float32)
    struct1, per_core1 = prep1(cfg, src, dst, deg_w)
    struct, per_core = prep(cfg, src, dst, deg_w)

    # bf16 x in padded-id order, plus one pair-layout per gap value g:
    # ag[s] = [x[s] | x[s+g]] so one 256B descriptor serves two edges
    xp = np.zeros((cfg.NPAD + cfg.G + 1, cfg.F1), ml_dtypes.bfloat16)
    xp[pid_map(cfg, np.arange(cfg.N))] = x.astype(ml_dtypes.bfloat16)
    ags = {}
    for g in range(cfg.G + 1):
        ags[f"ag{g}"] = np.concatenate(
            [xp[:cfg.NPAD], xp[g:cfg.NPAD + g]], axis=1)

    iota = np.arange(cfg.WIN, dtype=np.float32)
    shared = dict(
        W1b=W1.astype(ml_dtypes.bfloat16),
        W2b=W2.astype(ml_dtypes.bfloat16),
        b1d=b1.reshape(-1, 1).astype(np.float32),
        b2d=b2.reshape(-1, 1).astype(np.float32),
        iota16=np.tile(iota, (128, 1)).astype(ml_dtypes.bfloat16),
        ident16=np.eye(128, dtype=ml_dtypes.bfloat16),
        ident32=np.eye(128, dtype=np.float32),
        **ags,
    )
    in_maps = []
    for c in range(cfg.NC):
        m = dict(shared)
        m["idx"] = per_core[c]["idx"]
        m["dstrow"] = per_core[c]["dstrow"]
        m["wgt"] = per_core[c]["wgt"]
        m["idx1"] = per_core1[c]["idx1"]
        m["dstA"] = per_core1[c]["dstA"]
        m["wgtA"] = per_core1[c]["wgtA"]
        m["dstB"] = per_core1[c]["dstB"]
        m["wgtB"] = per_core1[c]["wgtB"]
        in_maps.append(m)

    key = (cfg.N, cfg.E, struct["totch"], tuple(struct["ch_lo"]),
           tuple(struct["ch_hi"]), struct1["totch1"], struct1["icols1"],
           tuple(tuple(tuple(cl) for cl in classes_h)
                 for ww in struct1["classes"] for classes_h in ww))
    if key not in _CACHE:
        _CACHE[key] = build_program(cfg, struct1, struct)
    return _CACHE[key], in_maps


def run(cfg, x, src, dst, W1, b1, W2, b2, trace=False, trace_kwargs=None):
    nc, in_maps = _get_program(cfg, x, src, dst, W1, b1, W2, b2)
    res = run_bass_kernel_spmd(nc, in_maps, core_ids=list(range(cfg.NC)),
                               trace=trace, **(trace_kwargs or {}))
    out = np.empty((cfg.N, cfg.F3), np.float32)
    for c in range(cfg.NC):
        o = np.asarray(res.results[c]["out"])  # [128, HALVES, F3]
        o = o.transpose(1, 0, 2).reshape(cfg.RPAD, cfg.F3)
        out[c * cfg.RPC:(c + 1) * cfg.RPC] = o[:cfg.RPC]
    return out, res


def kernel(x, src, dst, W1, b1, W2, b2):
    cfg = Cfg()
    out, _ = run(cfg, np.asarray(x, np.float32), np.asarray(src),
                 np.asarray(dst), np.asarray(W1, np.float32),
                 np.asarray(b1, np.float32), np.asarray(W2, np.float32),
                 np.asarray(b2, np.float32))
    return out



# revision 27
# speedup vs baseline: 1.3698x; 1.1271x over previous
# GraphSAGE 2-layer GNN on 8 TRN2 NeuronCores.
#
# Strategy (graph/data parallel, per sharding hint):
#   - dst-partition nodes across 8 cores (6250 rows each).
#   - Host: sort edges by (core, window, src), fold 1/(deg+eps) into per-edge
#     weights, build int16 gather-index streams + per-chunk one-hot metadata,
#     padded to a uniform max structure so all cores run one SPMD program.
#   - Device, per layer: bulk dma_gather of edge features (x rows fp32 /
#     h rows bf16) -> DVE builds weighted one-hot [128 edges x 256 rows] ->
#     TensorE segment-sum matmul into PSUM -> weight matmul -> bias(+relu)
#     on ACT -> PE transpose -> SBUF-resident h -> AllGather (bf16) ->
#     layer 2 -> batched log_softmax epilogue.
import sys

sys.path.insert(0, "/opt/trn_rl_repo")

import numpy as np
import ml_dtypes

import concourse.bass as bass
import concourse.bacc as bacc
import concourse.mybir as mybir
import concourse.tile as tile
from concourse.bass_utils import run_bass_kernel_spmd

F32 = mybir.dt.float32
F32R = mybir.dt.float32r
BF16 = mybir.dt.bfloat16
I16 = mybir.dt.int16


class Cfg:
    def __init__(self, N=50000, E=800000, F1=64, F2=128, F3=64, ncores=8,
                 win=256, lo_lim=32768, piece_chunks=70, pair_gap=3):
        self.N, self.E = N, E
        self.F1, self.F2, self.F3 = F1, F2, F3
        self.NC = ncores
        self.WIN = win
        self.LO = lo_lim
        self.PIECE = piece_chunks
        self.G = pair_gap                    # max src-gap for pair descriptors
        self.RPC = N // ncores               # rows per core
        assert self.RPC * ncores == N
        self.NWIN = -(-self.RPC // win)      # windows per core
        self.HALVES = -(-self.RPC // 128)    # 128-row halves per core
        self.RPAD = self.HALVES * 128        # padded rows per core
        self.NPAD = self.RPAD * ncores
        # window-aligned quarters, small last quarter so the final AG (the
        # one L2's first trigger waits on) has minimal data + minimal lag
        qw = sorted(set(min(w, self.NWIN) for w in
                        (0, 9, 17, self.NWIN - 1, self.NWIN)))
        self.QWIN = qw
        self.QN = len(qw) - 1
        self.QROWS = np.array(
            [min(w * win, self.RPAD) for w in qw], np.int64)


def pid_map(cfg, v):
    """Padded node id: rows grouped [quarter][core][row] so each quarter's
    allgather output is one contiguous flat slice of h_full."""
    v = np.asarray(v).astype(np.int64)
    c = v // cfg.RPC
    r = v % cfg.RPC
    q = np.searchsorted(cfg.QROWS, r, side="right") - 1
    qlen = cfg.QROWS[q + 1] - cfg.QROWS[q]
    return cfg.NC * cfg.QROWS[q] + c * qlen + (r - cfg.QROWS[q])


def prep1(cfg, src, dst, deg_w):
    """Layer-1 pair-packed structure.

    Edges are paired within (core, dst-window) when their padded source ids
    differ by g <= G; a pair is served by ONE 256B descriptor into A_g
    (A_g[s] = [x_bf16[s] | x_bf16[s+g]]).  Streams are per (window, half, g)
    with core-uniform slot counts; cores below the max fill slots with
    leftover single edges (degenerate pairs using only the first half).
    """
    NC, WIN, LO, RPC, G = cfg.NC, cfg.WIN, cfg.LO, cfg.RPC, cfg.G
    NWIN = cfg.NWIN
    pid = pid_map(cfg, src)
    core = dst // RPC

    # per (c, w): pair lists per (h, g), single lists per h
    pl_all = [[None] * NWIN for _ in range(NC)]
    sl_all = [[None] * NWIN for _ in range(NC)]
    npair = np.zeros((NC, NWIN, 2, G + 1), np.int64)
    nsing = np.zeros((NC, NWIN, 2), np.int64)
    for c in range(NC):
        m = core == c
        spc = pid[m]
        dl = (dst[m] - c * RPC).astype(np.int64)
        wvc = deg_w[dst[m]]
        wic = dl // WIN
        rowc = dl % WIN
        for w in range(NWIN):
            sel = wic == w
            s = spc[sel]
            r = rowc[sel]
            v = wvc[sel]
            o = np.argsort(s, kind="stable")
            s, r, v = s[o], r[o], v[o]
            pl = {}
            sl = ([], [])
            i, n = 0, len(s)
            while i < n:
                if i + 1 < n and s[i + 1] - s[i] <= G:
                    g = int(s[i + 1] - s[i])
                    h = int(s[i] >= LO)
                    pl.setdefault((h, g), []).append(
                        (s[i], r[i], v[i], r[i + 1], v[i + 1]))
                    i += 2
                else:
                    h = int(s[i] >= LO)
                    sl[h].append((s[i], r[i], v[i]))
                    i += 1
            pl_all[c][w] = pl
            sl_all[c][w] = sl
            for (h, g), lst in pl.items():
                npair[c, w, h, g] = len(lst)
            for h in (0, 1):
                nsing[c, w, h] = len(sl[h])

    base = npair.max(axis=0)                       # [NWIN, 2, G+1]
    slack = base[None] - npair
    absorbed = np.minimum(slack.sum(axis=3), nsing)
    leftover = nsing - absorbed
    extra = leftover.max(axis=0)                   # [NWIN, 2]
    nslot = base.copy()
    nslot[:, :, 0] += extra
    nslot16 = ((nslot + 15) // 16) * 16
    nchg_all = -(-nslot16 // 128)
    nchg_all[nslot16 == 0] = 0

    # class tables: per (w, h) list of (g, idx colbase, local chunk base,
    # nchg, nslot16, meta chunkbase)
    classes = [[[] for _ in range(2)] for _ in range(NWIN)]
    nch_wh = np.zeros((NWIN, 2), np.int64)
    icol = 0
    mchunk = 0
    for w in range(NWIN):
        for h in (0, 1):
            c0 = 0
            for g in range(G + 1):
                ns = int(nslot16[w, h, g])
                if ns == 0:
                    continue
                nchg = int(nchg_all[w, h, g])
                classes[w][h].append((g, icol, c0, nchg, ns, mchunk))
                icol += ns // 16
                c0 += nchg
                mchunk += nchg
            nch_wh[w, h] = c0
    totch1 = mchunk
    icols1 = icol
    nchmax = nch_wh.max(axis=0)                    # per half

    per_core = []
    for c in range(NC):
        idx1 = np.zeros(icols1 * 16, np.int16)
        dstA = np.full((totch1 * 128,), WIN, np.float32)
        wgtA = np.zeros((totch1 * 128,), np.float32)
        dstB = np.full((totch1 * 128,), WIN, np.float32)
        wgtB = np.zeros((totch1 * 128,), np.float32)
        for w in range(NWIN):
            pl = pl_all[c][w]
            sq = {0: list(sl_all[c][w][0]), 1: list(sl_all[c][w][1])}
            for h in (0, 1):
                for (g, cb, c0, nchg, ns, mb) in classes[w][h]:
                    pairs = pl.get((h, g), [])
                    k = len(pairs)
                    cap = ns
                    take = min(cap - k, len(sq[h]))
                    sing = [sq[h].pop() for _ in range(take)] if take > 0 \
                        else []
                    iv = np.zeros(ns, np.int16)
                    da = np.full(ns, WIN, np.float32)
                    wa = np.zeros(ns, np.float32)
                    db = np.full(ns, WIN, np.float32)
                    wb = np.zeros(ns, np.float32)
                    for j, (sp1, rA, vA, rB, vB) in enumerate(pairs):
                        iv[j] = sp1 - h * LO
                        da[j] = rA
                        wa[j] = vA
                        db[j] = rB
                        wb[j] = vB
                    for j, (sp1, rA, vA) in enumerate(sing):
                        iv[k + j] = sp1 - h * LO
                        da[k + j] = rA
                        wa[k + j] = vA
                    idx1[cb * 16:cb * 16 + ns] = iv
                    b = mb * 128
                    dstA[b:b + ns] = da
                    wgtA[b:b + ns] = wa
                    dstB[b:b + ns] = db
                    wgtB[b:b + ns] = wb
            assert not sq[0] and not sq[1], "singles left unplaced"
        idx16 = np.tile(idx1.reshape(-1, 16).T, (8, 1)).copy()
        per_core.append(dict(
            idx1=idx16,
            dstA=dstA.reshape(-1, 128).T.copy(),
            wgtA=wgtA.reshape(-1, 128).T.copy(),
            dstB=dstB.reshape(-1, 128).T.copy(),
            wgtB=wgtB.reshape(-1, 128).T.copy(),
        ))

    struct = dict(classes=classes, nch_wh=nch_wh, nchmax=nchmax,
                  totch1=totch1, icols1=icols1)
    return struct, per_core


def prep(cfg, src, dst, deg_w):
    """Build per-core gather/one-hot metadata with a core-uniform structure.

    Returns (struct, per_core) where struct has the shared max-shape info the
    program builder needs, and per_core the numpy arrays for in_maps.
    """
    NC, WIN, LO, RPC, RPAD = cfg.NC, cfg.WIN, cfg.LO, cfg.RPC, cfg.RPAD
    pid = pid_map(cfg, src)
    core = dst // RPC

    per_core_ed = []
    nlo = np.zeros((NC, cfg.NWIN), np.int64)
    nhi = np.zeros((NC, cfg.NWIN), np.int64)
    for c in range(NC):
        m = core == c
        sp = pid[m]
        dl = (dst[m] - c * RPC).astype(np.int64)
        wv = deg_w[dst[m]]
        wi = dl // WIN
        row = dl % WIN
        hi = (sp >= LO).astype(np.int64)
        order = np.lexsort((sp, hi, wi))
        sp, wv, wi, row, hi = sp[order], wv[order], wi[order], row[order], hi[order]
        per_core_ed.append((sp, wv, wi, row, hi))
        for w in range(cfg.NWIN):
            sel = wi == w
            nlo[c, w] = int((hi[sel] == 0).sum())
            nhi[c, w] = int(hi[sel].sum())

    # uniform chunk structure: per window, max #chunks across cores
    ch_lo = (-(-nlo.max(axis=0) // 128)).astype(np.int64)
    ch_hi = (-(-nhi.max(axis=0) // 128)).astype(np.int64)
    ch_lo = np.maximum(ch_lo, 1)
    ch_hi = np.maximum(ch_hi, 1)
    tot_lo, tot_hi = int(ch_lo.sum()), int(ch_hi.sum())
    totch = tot_lo + tot_hi

    # pieces: consecutive windows with total chunks <= PIECE
    pieces = []
    w0 = 0
    acc = 0
    for w in range(cfg.NWIN):
        cw = int(ch_lo[w] + ch_hi[w])
        assert cw <= cfg.PIECE, "single window exceeds piece budget"
        if acc + cw > cfg.PIECE:
            pieces.append((w0, w))
            w0, acc = w, 0
        acc += cw
    pieces.append((w0, cfg.NWIN))

    LB = np.concatenate([[0], np.cumsum(ch_lo)])   # lo-chunk base per window
    HB = np.concatenate([[0], np.cumsum(ch_hi)])
    MB = np.concatenate([[0], np.cumsum(ch_lo + ch_hi)])  # meta col base

    per_core = []
    for c in range(NC):
        sp, wv, wi, row, hi = per_core_ed[c]
        idx_lo = np.zeros(tot_lo * 128, np.int16)
        idx_hi = np.zeros(tot_hi * 128, np.int16)
        dstrow = np.full((totch * 128,), WIN, np.float32)  # sentinel row
        wgt = np.zeros((totch * 128,), np.float32)
        for w in range(cfg.NWIN):
            sel = wi == w
            sl = sel & (hi == 0)
            sh = sel & (hi == 1)
            klo, khi = int(sl.sum()), int(sh.sum())
            # lo stream
            b = LB[w] * 128
            idx_lo[b:b + klo] = sp[sl].astype(np.int16)
            # hi stream
            b = HB[w] * 128
            idx_hi[b:b + khi] = (sp[sh] - LO).astype(np.int16)
            # meta: lo chunks then hi chunks of this window
            b = MB[w] * 128
            dstrow[b:b + klo] = row[sl].astype(np.float32)
            wgt[b:b + klo] = wv[sl]
            b = (MB[w] + ch_lo[w]) * 128
            dstrow[b:b + khi] = row[sh].astype(np.float32)
            wgt[b:b + khi] = wv[sh]
        # idx arrays -> [16, n/16] interleave (idx i at [i%16, i//16])
        idx = np.concatenate([idx_lo, idx_hi])
        idx16 = np.tile(idx.reshape(-1, 16).T, (8, 1)).copy()
        per_core.append(dict(
            idx=idx16,
            dstrow=dstrow.reshape(-1, 128).T.copy(),
            wgt=wgt.reshape(-1, 128).T.copy(),
        ))

    struct = dict(ch_lo=ch_lo, ch_hi=ch_hi, tot_lo=tot_lo, tot_hi=tot_hi,
                  totch=totch, pieces=pieces, LB=LB, HB=HB, MB=MB)
    return struct, per_core


def build_program(cfg, struct1, struct):
    NC = cfg.NC
    F1, F2, F3, WIN = cfg.F1, cfg.F2, cfg.F3, cfg.WIN
    ch_lo, ch_hi = struct["ch_lo"], struct["ch_hi"]
    LB, HB, MB = struct["LB"], struct["HB"], struct["MB"]
    pieces = struct["pieces"]
    totch = struct["totch"]
    tot_lo = struct["tot_lo"]
    tot_hi = struct["tot_hi"]
    HALVES = cfg.HALVES
    classes1 = struct1["classes"]
    nch_wh = struct1["nch_wh"]
    nchmax1 = struct1["nchmax"]
    totch1 = struct1["totch1"]
    icols1 = struct1["icols1"]

    nc = bacc.Bacc("TRN2", target_bir_lowering=False, debug=False,
                   num_devices=NC)

    ag_d = [nc.dram_tensor(f"ag{g}", [cfg.NPAD, 2 * F1], BF16,
                           kind="ExternalInput") for g in range(cfg.G + 1)]
    W1b = nc.dram_tensor("W1b", [F1, F2], BF16, kind="ExternalInput")
    W2b = nc.dram_tensor("W2b", [F2, F3], BF16, kind="ExternalInput")
    b1d = nc.dram_tensor("b1d", [F2, 1], F32, kind="ExternalInput")
    b2d = nc.dram_tensor("b2d", [F3, 1], F32, kind="ExternalInput")
    iota16 = nc.dram_tensor("iota16", [128, WIN], BF16, kind="ExternalInput")
    ident16 = nc.dram_tensor("ident16", [128, 128], BF16, kind="ExternalInput")
    ident32 = nc.dram_tensor("ident32", [128, 128], F32, kind="ExternalInput")
    idx1_d = nc.dram_tensor("idx1", [128, icols1], I16, kind="ExternalInput")
    dstA_d = nc.dram_tensor("dstA", [128, totch1], F32, kind="ExternalInput")
    wgtA_d = nc.dram_tensor("wgtA", [128, totch1], F32, kind="ExternalInput")
    dstB_d = nc.dram_tensor("dstB", [128, totch1], F32, kind="ExternalInput")
    wgtB_d = nc.dram_tensor("wgtB", [128, totch1], F32, kind="ExternalInput")
    idx_d = nc.dram_tensor("idx", [128, totch * 8], I16, kind="ExternalInput")
    dstrow_d = nc.dram_tensor("dstrow", [128, totch], F32, kind="ExternalInput")
    wgt_d = nc.dram_tensor("wgt", [128, totch], F32, kind="ExternalInput")
    out_d = nc.dram_tensor("out", [128, HALVES, F3], F32, kind="ExternalOutput")

    with tile.TileContext(nc) as tc:
        with (
            tc.tile_pool(name="const", bufs=1) as cpool,
            tc.tile_pool(name="persist", bufs=1) as ppool,
            tc.tile_pool(name="dram", bufs=1, space="DRAM") as dpool,
        ):
            io16 = cpool.tile([128, WIN], BF16)
            nc.sync.dma_start(out=io16[:], in_=iota16[:])
            w1 = cpool.tile([F1, F2], BF16)
            nc.sync.dma_start(out=w1[:], in_=W1b[:])
            w2 = cpool.tile([F2, F3], BF16)
            nc.sync.dma_start(out=w2[:], in_=W2b[:])
            b1 = cpool.tile([F2, 1], F32)
            nc.sync.dma_start(out=b1[:], in_=b1d[:])
            b2 = cpool.tile([F3, 1], F32)
            nc.sync.dma_start(out=b2[:], in_=b2d[:])
            id16 = cpool.tile([128, 128], BF16)
            nc.sync.dma_start(out=id16[:], in_=ident16[:])
            id32 = cpool.tile([128, 128], F32)
            nc.sync.dma_start(out=id32[:], in_=ident32[:])
            idxs = cpool.tile([128, totch * 8], I16)
            idx_csz = -(-totch // 4) * 8
            for ic in range(4):
                a = ic * idx_csz
                b = min((ic + 1) * idx_csz, totch * 8)
                if b > a:
                    nc.sync.dma_start(out=idxs[:, a:b], in_=idx_d[:, a:b])
            dstrow = cpool.tile([128, totch], F32)
            nc.sync.dma_start(out=dstrow[:], in_=dstrow_d[:])
            wgt = cpool.tile([128, totch], F32)
            nc.sync.dma_start(out=wgt[:], in_=wgt_d[:])
            idx1s = cpool.tile([128, icols1], I16)
            i1_csz = -(-icols1 // 4)
            for ic in range(4):
                a = ic * i1_csz
                b = min((ic + 1) * i1_csz, icols1)
                if b > a:
                    nc.sync.dma_start(out=idx1s[:, a:b], in_=idx1_d[:, a:b])
            dstA = cpool.tile([128, totch1], F32)
            nc.sync.dma_start(out=dstA[:], in_=dstA_d[:])
            wgtA = cpool.tile([128, totch1], F32)
            nc.sync.dma_start(out=wgtA[:], in_=wgtA_d[:])
            dstB = cpool.tile([128, totch1], F32)
            nc.sync.dma_start(out=dstB[:], in_=dstB_d[:])
            wgtB = cpool.tile([128, totch1], F32)
            nc.sync.dma_start(out=wgtB[:], in_=wgtB_d[:])

            h_sb = ppool.tile([128, HALVES, F2], BF16)     # layer-1 output rows
            out_sb = ppool.tile([128, HALVES, F3], F32)    # layer-2 logits

            h_shard = dpool.tile([cfg.RPAD, F2], BF16)
            h_full = dpool.tile([cfg.NPAD, F2], BF16)

            def gather(dst_ap, in_ap, col0, n_chunks, elem):
                nc.gpsimd.dma_gather(
                    out_ap=dst_ap,
                    in_ap=in_ap,
                    idxs_ap=idxs[:, col0 * 8:(col0 + n_chunks) * 8],
                    num_idxs=n_chunks * 128,
                    num_idxs_reg=n_chunks * 128,
                    elem_size=elem,
                    single_packet=False,
                )

            def emit_ag(q):
                r0 = int(cfg.QROWS[q])
                r1 = int(cfg.QROWS[q + 1])
                nc.gpsimd.collective_compute(
                    "AllGather",
                    mybir.AluOpType.bypass,
                    replica_groups=[list(range(NC))],
                    ins=[h_shard[r0:r1, :].opt()],
                    outs=[h_full[NC * r0:NC * r1, :].opt()],
                )

            def run_layer1():
                with (
                    tc.tile_pool(name="g1", bufs=2) as gpool,
                    tc.tile_pool(name="oh1", bufs=32) as ohpool,
                    tc.tile_pool(name="ep1", bufs=2) as eppool,
                    tc.tile_pool(name="ps1", bufs=2, space="PSUM") as pspool,
                    tc.tile_pool(name="pt1", bufs=2, space="PSUM") as ptpool,
                ):
                    for w in range(cfg.NWIN):
                        tiles = [None, None]
                        for h in (0, 1):
                            cls = classes1[w][h]
                            if not cls:
                                continue
                            alloc = int(nchmax1[h]) if w < 2 \
                                else int(nch_wh[w, h])
                            t = gpool.tile([128, alloc, 2 * F1], BF16,
                                           tag=f"g1h{h}")
                            if w < 2:
                                # finite contents under every pad slot: a NaN
                                # bit pattern x 0-weight one-hot would poison
                                # the matmul accumulator
                                nc.gpsimd.memset(t[:], 0.0)
                            for (g, cb, c0, nchg, ns, mb) in cls:
                                src = ag_d[g][0:cfg.LO, :] if h == 0 \
                                    else ag_d[g][cfg.LO:cfg.NPAD, :]
                                nc.gpsimd.dma_gather(
                                    out_ap=t[:, c0:c0 + nchg, :],
                                    in_ap=src,
                                    idxs_ap=idx1s[:, cb:cb + ns // 16],
                                    num_idxs=ns,
                                    num_idxs_reg=ns,
                                    elem_size=2 * F1,
                                    single_packet=False,
                                )
                            tiles[h] = t
                        acc = pspool.tile([F1, WIN], F32, tag="acc")
                        nmm = 2 * int(nch_wh[w, 0] + nch_wh[w, 1])
                        k = 0
                        for h in (0, 1):
                            for (g, cb, c0, nchg, ns, mb) in classes1[w][h]:
                                for ck in range(nchg):
                                    mc = mb + ck
                                    ohA = ohpool.tile([128, WIN], BF16,
                                                      tag="oh")
                                    nc.vector.tensor_scalar(
                                        out=ohA[:], in0=io16[:],
                                        scalar1=dstA[:, mc:mc + 1],
                                        scalar2=wgtA[:, mc:mc + 1],
                                        op0=mybir.AluOpType.is_equal,
                                        op1=mybir.AluOpType.mult)
                                    nc.tensor.matmul(
                                        out=acc[:],
                                        lhsT=tiles[h][:, c0 + ck, 0:F1],
                                        rhs=ohA[:],
                                        start=(k == 0), stop=(k == nmm - 1))
                                    k += 1
                                    ohB = ohpool.tile([128, WIN], BF16,
                                                      tag="oh")
                                    nc.vector.tensor_scalar(
                                        out=ohB[:], in0=io16[:],
                                        scalar1=dstB[:, mc:mc + 1],
                                        scalar2=wgtB[:, mc:mc + 1],
                                        op0=mybir.AluOpType.is_equal,
                                        op1=mybir.AluOpType.mult)
                                    nc.tensor.matmul(
                                        out=acc[:],
                                        lhsT=tiles[h][:, c0 + ck, F1:2 * F1],
                                        rhs=ohB[:],
                                        start=(k == 0), stop=(k == nmm - 1))
                                    k += 1
                        # window epilogue
                        mbf = eppool.tile([F1, WIN], BF16, tag="mbf")
                        nc.scalar.activation(
                            out=mbf[:], in_=acc[:],
                            func=mybir.ActivationFunctionType.Identity)
                        z = ptpool.tile([F2, WIN], F32, tag="z")
                        nc.tensor.matmul(out=z[:], lhsT=w1[:],
                                         rhs=mbf[:], start=True,
                                         stop=True)
                        ht = eppool.tile([F2, WIN], BF16, tag="ht")
                        nc.scalar.activation(
                            out=ht[:], in_=z[:],
                            func=mybir.ActivationFunctionType.Relu,
                            bias=b1[:, 0:1])
                        for hf in range(WIN // 128):
                            hh = w * (WIN // 128) + hf
                            if hh >= HALVES:
                                continue
                            tp = ptpool.tile([128, 128], BF16, tag="tp")
                            nc.tensor.transpose(
                                out=tp[:],
                                in_=ht[:, hf * 128:(hf + 1) * 128],
                                identity=id16[:])
                            nc.scalar.activation(
                                out=h_sb[:, hh, :], in_=tp[:],
                                func=mybir.ActivationFunctionType.Identity)
                        # ship this window's h rows to DRAM now so the
                        # allgather input is ready as soon as L1 ends
                        hh0 = w * (WIN // 128)
                        hh1 = min(hh0 + WIN // 128, HALVES)
                        if hh1 > hh0:
                            nc.sync.dma_start(
                                out=h_shard[:].rearrange(
                                    "(hh p) f -> p hh f", p=128)[:, hh0:hh1, :],
                                in_=h_sb[:, hh0:hh1, :])
                        # fire quarter AllGathers mid-stream, LAG windows
                        # after the quarter's last window was issued, so
                        # the Pool engine never stalls waiting for compute
                        for q in range(cfg.QN):
                            if w + 1 == min(cfg.QWIN[q + 1] + 2,
                                            cfg.NWIN) and \
                                    cfg.QWIN[q + 1] + 2 <= cfg.NWIN:
                                emit_ag(q)

            def run_layer(layer):
                assert layer == 2
                elem, gdt = F2, BF16
                src_lo = h_full[0:cfg.LO, :]
                src_hi = h_full[cfg.LO:cfg.NPAD, :]
                with (
                    tc.tile_pool(name=f"g{layer}", bufs=2) as gpool,
                    tc.tile_pool(name=f"oh{layer}", bufs=32) as ohpool,
                    tc.tile_pool(name=f"ep{layer}", bufs=2) as eppool,
                    tc.tile_pool(name=f"ps{layer}", bufs=2, space="PSUM") as pspool,
                    tc.tile_pool(name=f"pt{layer}", bufs=2, space="PSUM") as ptpool,
                ):
                    piece_of = {}
                    g_lo_t = {}
                    g_hi_t = {}
                    for pi, (w0, w1_) in enumerate(pieces):
                        for w in range(w0, w1_):
                            piece_of[w] = pi

                    def need_piece(pi):
                        if pi in g_lo_t:
                            return
                        w0, w1_ = pieces[pi]
                        ncl = int(LB[w1_] - LB[w0])
                        nch = int(HB[w1_] - HB[w0])
                        g_lo = gpool.tile([128, ncl, elem], gdt, tag="glo")
                        g_hi = gpool.tile([128, nch, elem], gdt, tag="ghi")
                        gather(g_lo[:, :, :], src_lo, int(LB[w0]), ncl, elem)
                        gather(g_hi[:, :, :], src_hi, int(tot_lo + HB[w0]),
                               nch, elem)
                        g_lo_t[pi] = g_lo
                        g_hi_t[pi] = g_hi

                    def glo_chunk(w, gc):
                        pi = piece_of[w]
                        need_piece(pi)
                        return g_lo_t[pi][:, gc - int(LB[pieces[pi][0]]), :]

                    def ghi_chunk(w, gc):
                        pi = piece_of[w]
                        need_piece(pi)
                        return g_hi_t[pi][:, gc - int(HB[pieces[pi][0]]), :]

                    for w in range(cfg.NWIN):
                        nl, nh = int(ch_lo[w]), int(ch_hi[w])
                        acc = pspool.tile([F2, WIN], F32, tag="acc")
                        tot = nl + nh
                        for k in range(tot):
                            if k < nl:
                                g = glo_chunk(w, int(LB[w]) + k)
                            else:
                                g = ghi_chunk(w, int(HB[w]) + k - nl)
                            mc = int(MB[w]) + k
                            oh = ohpool.tile([128, WIN], BF16, tag="oh")
                            nc.vector.tensor_scalar(
                                out=oh[:], in0=io16[:],
                                scalar1=dstrow[:, mc:mc + 1],
                                scalar2=wgt[:, mc:mc + 1],
                                op0=mybir.AluOpType.is_equal,
                                op1=mybir.AluOpType.mult)
                            nc.tensor.matmul(
                                out=acc[:], lhsT=g, rhs=oh[:],
                                start=(k == 0), stop=(k == tot - 1))
                        # window epilogue
                        if True:
                            mbf = eppool.tile([F2, WIN], BF16, tag="mbf")
                            nc.scalar.activation(
                                out=mbf[:], in_=acc[:],
                                func=mybir.ActivationFunctionType.Identity)
                            z = ptpool.tile([F3, WIN], F32, tag="z")
                            nc.tensor.matmul(out=z[:], lhsT=w2[:],
                                             rhs=mbf[:], start=True,
                                             stop=True)
                            o2 = eppool.tile([128, WIN], F32, tag="ht")
                            nc.scalar.activation(
                                out=o2[0:F3, :], in_=z[:],
                                func=mybir.ActivationFunctionType.Identity,
                                bias=b2[:, 0:1])
                            for hf in range(WIN // 128):
                                hh = w * (WIN // 128) + hf
                                if hh >= HALVES:
                                    continue
                                tp = ptpool.tile([128, 128], F32, tag="tp")
                                nc.tensor.transpose(
                                    out=tp[:],
                                    in_=o2[:, hf * 128:(hf + 1) * 128],
                                    identity=id32[:])
                                nc.scalar.activation(
                                    out=out_sb[:, hh, :], in_=tp[:, 0:F3],
                                    func=mybir.ActivationFunctionType.Identity)
                            # quarter boundary: log_softmax + ship output rows
                            for q in range(cfg.QN):
                                q0 = int(cfg.QROWS[q]) // 128
                                q1 = int(cfg.QROWS[q + 1]) // 128
                                if w == (q1 - 1) // (WIN // 128):
                                    nh = q1 - q0
                                    nhx = (int(cfg.QROWS[-1]) -
                                           int(cfg.QROWS[-2])) // 128
                                    nhx = max(nhx, nh)
                                    sl = out_sb[:, q0:q1, :]
                                    mx = eppool.tile([128, nhx, 1], F32,
                                                     tag="smx")
                                    nc.vector.reduce_max(
                                        out=mx[:, 0:nh, :], in_=sl,
                                        axis=mybir.AxisListType.X)
                                    nc.vector.tensor_tensor(
                                        out=sl, in0=sl,
                                        in1=mx[:, 0:nh, :].to_broadcast(
                                            [128, nh, F3]),
                                        op=mybir.AluOpType.subtract)
                                    ex = eppool.tile([128, nhx, F3], F32,
                                                     tag="sex")
                                    nc.scalar.activation(
                                        out=ex[:, 0:nh, :], in_=sl,
                                        func=mybir.ActivationFunctionType.Exp)
                                    sm = eppool.tile([128, nhx, 1], F32,
                                                     tag="ssm")
                                    nc.vector.reduce_sum(
                                        out=sm[:, 0:nh, :], in_=ex[:, 0:nh, :],
                                        axis=mybir.AxisListType.X)
                                    ls = eppool.tile([128, nhx, 1], F32,
                                                     tag="sls")
                                    nc.scalar.activation(
                                        out=ls[:, 0:nh, :], in_=sm[:, 0:nh, :],
                                        func=mybir.ActivationFunctionType.Ln)
                                    nc.vector.tensor_tensor(
                                        out=sl, in0=sl,
                                        in1=ls[:, 0:nh, :].to_broadcast(
                                            [128, nh, F3]),
                                        op=mybir.AluOpType.subtract)
                                    nc.sync.dma_start(
                                        out=out_d[:, q0:q1, :], in_=sl)

            import os
            stage = os.environ.get("K_STAGE", "full")
            run_layer1()
            if stage != "l1":
                # quarters whose (end + LAG) passed NWIN fire here, after the
                # last L1 window was issued
                for q in range(cfg.QN):
                    if cfg.QWIN[q + 1] + 2 > cfg.NWIN:
                        emit_ag(q)
            if stage == "full":
                run_layer(2)

    nc.compile()
    return nc


_CACHE = {}


def _get_program(cfg, x, src, dst, W1, b1, W2, b2):
    deg = np.bincount(dst, minlength=cfg.N).astype(np.float64)
    deg_w = (1.0 / (deg + 1e-6)).astype(np.float32)
    struct1, per_core1 = prep1(cfg, src, dst, deg_w)
    struct, per_core = prep(cfg, src, dst, deg_w)

    # bf16 x in padded-id order, plus one pair-layout per gap value g:
    # ag[s] = [x[s] | x[s+g]] so one 256B descriptor serves two edges
    xp = np.zeros((cfg.NPAD + cfg.G + 1, cfg.F1), ml_dtypes.bfloat16)
    xp[pid_map(cfg, np.arange(cfg.N))] = x.astype(ml_dtypes.bfloat16)
    ags = {}
    for g in range(cfg.G + 1):
        ags[f"ag{g}"] = np.concatenate(
            [xp[:cfg.NPAD], xp[g:cfg.NPAD + g]], axis=1)

    iota = np.arange(cfg.WIN, dtype=np.float32)
    shared = dict(
        W1b=W1.astype(ml_dtypes.bfloat16),
        W2b=W2.astype(ml_dtypes.bfloat16),
        b1d=b1.reshape(-1, 1).astype(np.float32),
        b2d=b2.reshape(-1, 1).astype(np.float32),
        iota16=np.tile(iota, (128, 1)).astype(ml_dtypes.bfloat16),
        ident16=np.eye(128, dtype=ml_dtypes.bfloat16),
        ident32=np.eye(128, dtype=np.float32),
        **ags,
    )
    in_maps = []
    for c in range(cfg.NC):
        m = dict(shared)
        m["idx"] = per_core[c]["idx"]
        m["dstrow"] = per_core[c]["dstrow"]
        m["wgt"] = per_core[c]["wgt"]
        m["idx1"] = per_core1[c]["idx1"]
        m["dstA"] = per_core1[c]["dstA"]
        m["wgtA"] = per_core1[c]["wgtA"]
        m["dstB"] = per_core1[c]["dstB"]
        m["wgtB"] = per_core1[c]["wgtB"]
        in_maps.append(m)

    key = (cfg.N, cfg.E, struct["totch"], tuple(struct["ch_lo"]),
           tuple(struct["ch_hi"]), struct1["totch1"], struct1["icols1"],
           tuple(tuple(tuple(cl) for cl in classes_h)
                 for ww in struct1["classes"] for classes_h in ww))
    if key not in _CACHE:
        _CACHE[key] = build_program(cfg, struct1, struct)
    return _CACHE[key], in_maps


def run(cfg, x, src, dst, W1, b1, W2, b2, trace=False, trace_kwargs=None):
    nc, in_maps = _get_program(cfg, x, src, dst, W1, b1, W2, b2)
    res = run_bass_kernel_spmd(nc, in_maps, core_ids=list(range(cfg.NC)),
                               trace=trace, **(trace_kwargs or {}))
    out = np.empty((cfg.N, cfg.F3), np.float32)
    for c in range(cfg.NC):
        o = np.asarray(res.results[c]["out"])  # [128, HALVES, F3]
        o = o.transpose(1, 0, 2).reshape(cfg.RPAD, cfg.F3)
        out[c * cfg.RPC:(c + 1) * cfg.RPC] = o[:cfg.RPC]
    return out, res


def kernel(x, src, dst, W1, b1, W2, b2):
    cfg = Cfg()
    out, _ = run(cfg, np.asarray(x, np.float32), np.asarray(src),
                 np.asarray(dst), np.asarray(W1, np.float32),
                 np.asarray(b1, np.float32), np.asarray(W2, np.float32),
                 np.asarray(b2, np.float32))
    return out



# revision 30
# speedup vs baseline: 1.3975x; 1.0203x over previous
# GraphSAGE 2-layer GNN on 8 TRN2 NeuronCores.
#
# Strategy (graph/data parallel, per sharding hint):
#   - dst-partition nodes across 8 cores (6250 rows each).
#   - Host: sort edges by (core, window, src), fold 1/(deg+eps) into per-edge
#     weights, build int16 gather-index streams + per-chunk one-hot metadata,
#     padded to a uniform max structure so all cores run one SPMD program.
#   - Device, per layer: bulk dma_gather of edge features (x rows fp32 /
#     h rows bf16) -> DVE builds weighted one-hot [128 edges x 256 rows] ->
#     TensorE segment-sum matmul into PSUM -> weight matmul -> bias(+relu)
#     on ACT -> PE transpose -> SBUF-resident h -> AllGather (bf16) ->
#     layer 2 -> batched log_softmax epilogue.
import sys

sys.path.insert(0, "/opt/trn_rl_repo")

import numpy as np
import ml_dtypes

import concourse.bass as bass
import concourse.bacc as bacc
import concourse.mybir as mybir
import concourse.tile as tile
from concourse.bass_utils import run_bass_kernel_spmd

F32 = mybir.dt.float32
F32R = mybir.dt.float32r
F16 = mybir.dt.float16
BF16 = mybir.dt.bfloat16
I16 = mybir.dt.int16


class Cfg:
    def __init__(self, N=50000, E=800000, F1=64, F2=128, F3=64, ncores=8,
                 win=512, lo_lim=32768, piece_chunks=96, pair_gap=3):
        self.N, self.E = N, E
        self.F1, self.F2, self.F3 = F1, F2, F3
        self.NC = ncores
        self.WIN = win
        self.LO = lo_lim
        self.PIECE = piece_chunks
        self.G = pair_gap                    # max src-gap for pair descriptors
        self.RPC = N // ncores               # rows per core
        assert self.RPC * ncores == N
        self.NWIN = -(-self.RPC // win)      # windows per core
        self.HALVES = -(-self.RPC // 128)    # 128-row halves per core
        self.RPAD = self.HALVES * 128        # padded rows per core
        self.NPAD = self.RPAD * ncores
        # window-aligned quarters, small last quarter so the final AG (the
        # one L2's first trigger waits on) has minimal data + minimal lag
        qw = sorted(set(min(w, self.NWIN) for w in
                        (0, 9, 17, self.NWIN - 1, self.NWIN)))
        self.QWIN = qw
        self.QN = len(qw) - 1
        self.QROWS = np.array(
            [min(w * win, self.RPAD) for w in qw], np.int64)


def pid_map(cfg, v):
    """Padded node id: rows grouped [quarter][core][row] so each quarter's
    allgather output is one contiguous flat slice of h_full."""
    v = np.asarray(v).astype(np.int64)
    c = v // cfg.RPC
    r = v % cfg.RPC
    q = np.searchsorted(cfg.QROWS, r, side="right") - 1
    qlen = cfg.QROWS[q + 1] - cfg.QROWS[q]
    return cfg.NC * cfg.QROWS[q] + c * qlen + (r - cfg.QROWS[q])


def prep1(cfg, src, dst, deg_w):
    """Layer-1 pair-packed structure.

    Edges are paired within (core, dst-window) when their padded source ids
    differ by g <= G; a pair is served by ONE 256B descriptor into A_g
    (A_g[s] = [x_bf16[s] | x_bf16[s+g]]).  Streams are per (window, half, g)
    with core-uniform slot counts; cores below the max fill slots with
    leftover single edges (degenerate pairs using only the first half).
    """
    NC, WIN, LO, RPC, G = cfg.NC, cfg.WIN, cfg.LO, cfg.RPC, cfg.G
    NWIN = cfg.NWIN
    pid = pid_map(cfg, src)
    core = dst // RPC

    # per (c, w): pair lists per (h, g), single lists per h
    pl_all = [[None] * NWIN for _ in range(NC)]
    sl_all = [[None] * NWIN for _ in range(NC)]
    npair = np.zeros((NC, NWIN, 2, G + 1), np.int64)
    nsing = np.zeros((NC, NWIN, 2), np.int64)
    for c in range(NC):
        m = core == c
        spc = pid[m]
        dl = (dst[m] - c * RPC).astype(np.int64)
        wvc = deg_w[dst[m]]
        wic = dl // WIN
        rowc = dl % WIN
        for w in range(NWIN):
            sel = wic == w
            s = spc[sel]
            r = rowc[sel]
            v = wvc[sel]
            o = np.argsort(s, kind="stable")
            s, r, v = s[o], r[o], v[o]
            pl = {}
            sl = ([], [])
            i, n = 0, len(s)
            while i < n:
                if i + 1 < n and s[i + 1] - s[i] <= G:
                    g = int(s[i + 1] - s[i])
                    h = int(s[i] >= LO)
                    pl.setdefault((h, g), []).append(
                        (s[i], r[i], v[i], r[i + 1], v[i + 1]))
                    i += 2
                else:
                    h = int(s[i] >= LO)
                    sl[h].append((s[i], r[i], v[i]))
                    i += 1
            pl_all[c][w] = pl
            sl_all[c][w] = sl
            for (h, g), lst in pl.items():
                npair[c, w, h, g] = len(lst)
            for h in (0, 1):
                nsing[c, w, h] = len(sl[h])

    base = npair.max(axis=0)                       # [NWIN, 2, G+1]
    slack = base[None] - npair
    absorbed = np.minimum(slack.sum(axis=3), nsing)
    leftover = nsing - absorbed
    extra = leftover.max(axis=0)                   # [NWIN, 2]
    nslot = base.copy()
    nslot[:, :, 0] += extra
    nslot16 = ((nslot + 15) // 16) * 16
    nchg_all = -(-nslot16 // 128)
    nchg_all[nslot16 == 0] = 0

    # class tables: per (w, h) list of (g, idx colbase, local chunk base,
    # nchg, nslot16, meta chunkbase)
    classes = [[[] for _ in range(2)] for _ in range(NWIN)]
    nch_wh = np.zeros((NWIN, 2), np.int64)
    icol = 0
    mchunk = 0
    for w in range(NWIN):
        for h in (0, 1):
            c0 = 0
            for g in range(G + 1):
                ns = int(nslot16[w, h, g])
                if ns == 0:
                    continue
                nchg = int(nchg_all[w, h, g])
                classes[w][h].append((g, icol, c0, nchg, ns, mchunk))
                icol += ns // 16
                c0 += nchg
                mchunk += nchg
            nch_wh[w, h] = c0
    totch1 = mchunk
    icols1 = icol
    nchmax = nch_wh.max(axis=0)                    # per half

    per_core = []
    for c in range(NC):
        idx1 = np.zeros(icols1 * 16, np.int16)
        dstA = np.full((totch1 * 128,), WIN, np.float32)
        wgtA = np.zeros((totch1 * 128,), np.float32)
        dstB = np.full((totch1 * 128,), WIN, np.float32)
        wgtB = np.zeros((totch1 * 128,), np.float32)
        for w in range(NWIN):
            pl = pl_all[c][w]
            sq = {0: list(sl_all[c][w][0]), 1: list(sl_all[c][w][1])}
            for h in (0, 1):
                for (g, cb, c0, nchg, ns, mb) in classes[w][h]:
                    pairs = pl.get((h, g), [])
                    k = len(pairs)
                    cap = ns
                    take = min(cap - k, len(sq[h]))
                    sing = [sq[h].pop() for _ in range(take)] if take > 0 \
                        else []
                    iv = np.zeros(ns, np.int16)
                    da = np.full(ns, WIN, np.float32)
                    wa = np.zeros(ns, np.float32)
                    db = np.full(ns, WIN, np.float32)
                    wb = np.zeros(ns, np.float32)
                    for j, (sp1, rA, vA, rB, vB) in enumerate(pairs):
                        iv[j] = sp1 - h * LO
                        da[j] = rA
                        wa[j] = vA
                        db[j] = rB
                        wb[j] = vB
                    for j, (sp1, rA, vA) in enumerate(sing):
                        iv[k + j] = sp1 - h * LO
                        da[k + j] = rA
                        wa[k + j] = vA
                    idx1[cb * 16:cb * 16 + ns] = iv
                    b = mb * 128
                    dstA[b:b + ns] = da
                    wgtA[b:b + ns] = wa
                    dstB[b:b + ns] = db
                    wgtB[b:b + ns] = wb
            assert not sq[0] and not sq[1], "singles left unplaced"
        idx16 = np.tile(idx1.reshape(-1, 16).T, (8, 1)).copy()
        per_core.append(dict(
            idx1=idx16,
            dstA=dstA.reshape(-1, 128).T.copy(),
            wgtA=wgtA.reshape(-1, 128).T.copy(),
            dstB=dstB.reshape(-1, 128).T.copy(),
            wgtB=wgtB.reshape(-1, 128).T.copy(),
        ))

    struct = dict(classes=classes, nch_wh=nch_wh, nchmax=nchmax,
                  totch1=totch1, icols1=icols1)
    return struct, per_core


def prep(cfg, src, dst, deg_w):
    """Build per-core gather/one-hot metadata with a core-uniform structure.

    Returns (struct, per_core) where struct has the shared max-shape info the
    program builder needs, and per_core the numpy arrays for in_maps.
    """
    NC, WIN, LO, RPC, RPAD = cfg.NC, cfg.WIN, cfg.LO, cfg.RPC, cfg.RPAD
    pid = pid_map(cfg, src)
    core = dst // RPC

    per_core_ed = []
    nlo = np.zeros((NC, cfg.NWIN), np.int64)
    nhi = np.zeros((NC, cfg.NWIN), np.int64)
    for c in range(NC):
        m = core == c
        sp = pid[m]
        dl = (dst[m] - c * RPC).astype(np.int64)
        wv = deg_w[dst[m]]
        wi = dl // WIN
        row = dl % WIN
        hi = (sp >= LO).astype(np.int64)
        order = np.lexsort((sp, hi, wi))
        sp, wv, wi, row, hi = sp[order], wv[order], wi[order], row[order], hi[order]
        per_core_ed.append((sp, wv, wi, row, hi))
        for w in range(cfg.NWIN):
            sel = wi == w
            nlo[c, w] = int((hi[sel] == 0).sum())
            nhi[c, w] = int(hi[sel].sum())

    # uniform chunk structure: per window, max #chunks across cores
    ch_lo = (-(-nlo.max(axis=0) // 128)).astype(np.int64)
    ch_hi = (-(-nhi.max(axis=0) // 128)).astype(np.int64)
    ch_lo = np.maximum(ch_lo, 1)
    ch_hi = np.maximum(ch_hi, 1)
    tot_lo, tot_hi = int(ch_lo.sum()), int(ch_hi.sum())
    totch = tot_lo + tot_hi

    # pieces: consecutive windows with total chunks <= PIECE
    pieces = []
    w0 = 0
    acc = 0
    for w in range(cfg.NWIN):
        cw = int(ch_lo[w] + ch_hi[w])
        assert cw <= cfg.PIECE, "single window exceeds piece budget"
        if acc + cw > cfg.PIECE:
            pieces.append((w0, w))
            w0, acc = w, 0
        acc += cw
    pieces.append((w0, cfg.NWIN))

    LB = np.concatenate([[0], np.cumsum(ch_lo)])   # lo-chunk base per window
    HB = np.concatenate([[0], np.cumsum(ch_hi)])
    MB = np.concatenate([[0], np.cumsum(ch_lo + ch_hi)])  # meta col base

    per_core = []
    for c in range(NC):
        sp, wv, wi, row, hi = per_core_ed[c]
        idx_lo = np.zeros(tot_lo * 128, np.int16)
        idx_hi = np.zeros(tot_hi * 128, np.int16)
        dstrow = np.full((totch * 128,), WIN, np.float32)  # sentinel row
        wgt = np.zeros((totch * 128,), np.float32)
        for w in range(cfg.NWIN):
            sel = wi == w
            sl = sel & (hi == 0)
            sh = sel & (hi == 1)
            klo, khi = int(sl.sum()), int(sh.sum())
            # lo stream
            b = LB[w] * 128
            idx_lo[b:b + klo] = sp[sl].astype(np.int16)
            # hi stream
            b = HB[w] * 128
            idx_hi[b:b + khi] = (sp[sh] - LO).astype(np.int16)
            # meta: lo chunks then hi chunks of this window
            b = MB[w] * 128
            dstrow[b:b + klo] = row[sl].astype(np.float32)
            wgt[b:b + klo] = wv[sl]
            b = (MB[w] + ch_lo[w]) * 128
            dstrow[b:b + khi] = row[sh].astype(np.float32)
            wgt[b:b + khi] = wv[sh]
        # idx arrays -> [16, n/16] interleave (idx i at [i%16, i//16])
        idx = np.concatenate([idx_lo, idx_hi])
        idx16 = np.tile(idx.reshape(-1, 16).T, (8, 1)).copy()
        per_core.append(dict(
            idx=idx16,
            dstrow=dstrow.reshape(-1, 128).T.copy(),
            wgt=wgt.reshape(-1, 128).T.copy(),
        ))

    struct = dict(ch_lo=ch_lo, ch_hi=ch_hi, tot_lo=tot_lo, tot_hi=tot_hi,
                  totch=totch, pieces=pieces, LB=LB, HB=HB, MB=MB)
    return struct, per_core


def build_program(cfg, struct1, struct):
    NC = cfg.NC
    F1, F2, F3, WIN = cfg.F1, cfg.F2, cfg.F3, cfg.WIN
    ch_lo, ch_hi = struct["ch_lo"], struct["ch_hi"]
    LB, HB, MB = struct["LB"], struct["HB"], struct["MB"]
    pieces = struct["pieces"]
    totch = struct["totch"]
    tot_lo = struct["tot_lo"]
    tot_hi = struct["tot_hi"]
    HALVES = cfg.HALVES
    classes1 = struct1["classes"]
    nch_wh = struct1["nch_wh"]
    nchmax1 = struct1["nchmax"]
    totch1 = struct1["totch1"]
    icols1 = struct1["icols1"]

    nc = bacc.Bacc("TRN2", target_bir_lowering=False, debug=False,
                   num_devices=NC)

    ag_d = [nc.dram_tensor(f"ag{g}", [cfg.NPAD, 2 * F1], BF16,
                           kind="ExternalInput") for g in range(cfg.G + 1)]
    W1b = nc.dram_tensor("W1b", [F1, F2], BF16, kind="ExternalInput")
    W2b = nc.dram_tensor("W2b", [F2, F3], BF16, kind="ExternalInput")
    b1d = nc.dram_tensor("b1d", [F2, 1], F32, kind="ExternalInput")
    b2d = nc.dram_tensor("b2d", [F3, 1], F32, kind="ExternalInput")
    iota16 = nc.dram_tensor("iota16", [128, WIN], F16, kind="ExternalInput")
    ident16 = nc.dram_tensor("ident16", [128, 128], BF16, kind="ExternalInput")
    ident32 = nc.dram_tensor("ident32", [128, 128], F32, kind="ExternalInput")
    idx1_d = nc.dram_tensor("idx1", [128, icols1], I16, kind="ExternalInput")
    dstA_d = nc.dram_tensor("dstA", [128, totch1], F32, kind="ExternalInput")
    wgtA_d = nc.dram_tensor("wgtA", [128, totch1], F32, kind="ExternalInput")
    dstB_d = nc.dram_tensor("dstB", [128, totch1], F32, kind="ExternalInput")
    wgtB_d = nc.dram_tensor("wgtB", [128, totch1], F32, kind="ExternalInput")
    idx_d = nc.dram_tensor("idx", [128, totch * 8], I16, kind="ExternalInput")
    dstrow_d = nc.dram_tensor("dstrow", [128, totch], F32, kind="ExternalInput")
    wgt_d = nc.dram_tensor("wgt", [128, totch], F32, kind="ExternalInput")
    out_d = nc.dram_tensor("out", [128, HALVES, F3], F32, kind="ExternalOutput")

    with tile.TileContext(nc) as tc:
        with (
            tc.tile_pool(name="const", bufs=1) as cpool,
            tc.tile_pool(name="persist", bufs=1) as ppool,
            tc.tile_pool(name="dram", bufs=1, space="DRAM") as dpool,
        ):
            io16 = cpool.tile([128, WIN], F16)
            nc.sync.dma_start(out=io16[:], in_=iota16[:])
            w1 = cpool.tile([F1, F2], BF16)
            nc.sync.dma_start(out=w1[:], in_=W1b[:])
            w2 = cpool.tile([F2, F3], BF16)
            nc.sync.dma_start(out=w2[:], in_=W2b[:])
            b1 = cpool.tile([F2, 1], F32)
            nc.sync.dma_start(out=b1[:], in_=b1d[:])
            b2 = cpool.tile([F3, 1], F32)
            nc.sync.dma_start(out=b2[:], in_=b2d[:])
            id16 = cpool.tile([128, 128], BF16)
            nc.sync.dma_start(out=id16[:], in_=ident16[:])
            id32 = cpool.tile([128, 128], F32)
            nc.sync.dma_start(out=id32[:], in_=ident32[:])
            idxs = cpool.tile([128, totch * 8], I16)
            idx_csz = -(-totch // 4) * 8
            for ic in range(4):
                a = ic * idx_csz
                b = min((ic + 1) * idx_csz, totch * 8)
                if b > a:
                    nc.sync.dma_start(out=idxs[:, a:b], in_=idx_d[:, a:b])
            dstrow = cpool.tile([128, totch], F32)
            nc.sync.dma_start(out=dstrow[:], in_=dstrow_d[:])
            wgt = cpool.tile([128, totch], F32)
            nc.sync.dma_start(out=wgt[:], in_=wgt_d[:])
            idx1s = cpool.tile([128, icols1], I16)
            i1_csz = -(-icols1 // 4)
            for ic in range(4):
                a = ic * i1_csz
                b = min((ic + 1) * i1_csz, icols1)
                if b > a:
                    nc.sync.dma_start(out=idx1s[:, a:b], in_=idx1_d[:, a:b])
            dstA = cpool.tile([128, totch1], F32)
            nc.sync.dma_start(out=dstA[:], in_=dstA_d[:])
            wgtA = cpool.tile([128, totch1], F32)
            nc.sync.dma_start(out=wgtA[:], in_=wgtA_d[:])
            dstB = cpool.tile([128, totch1], F32)
            nc.sync.dma_start(out=dstB[:], in_=dstB_d[:])
            wgtB = cpool.tile([128, totch1], F32)
            nc.sync.dma_start(out=wgtB[:], in_=wgtB_d[:])

            h_sb = ppool.tile([128, HALVES, F2], BF16)     # layer-1 output rows
            out_sb = ppool.tile([128, HALVES, F3], F32)    # layer-2 logits

            h_shard = dpool.tile([cfg.RPAD, F2], BF16)
            h_full = dpool.tile([cfg.NPAD, F2], BF16)

            def gather(dst_ap, in_ap, col0, n_chunks, elem):
                nc.gpsimd.dma_gather(
                    out_ap=dst_ap,
                    in_ap=in_ap,
                    idxs_ap=idxs[:, col0 * 8:(col0 + n_chunks) * 8],
                    num_idxs=n_chunks * 128,
                    num_idxs_reg=n_chunks * 128,
                    elem_size=elem,
                    single_packet=False,
                )

            def emit_ag(q):
                r0 = int(cfg.QROWS[q])
                r1 = int(cfg.QROWS[q + 1])
                nc.gpsimd.collective_compute(
                    "AllGather",
                    mybir.AluOpType.bypass,
                    replica_groups=[list(range(NC))],
                    ins=[h_shard[r0:r1, :].opt()],
                    outs=[h_full[NC * r0:NC * r1, :].opt()],
                )

            def run_layer1():
                with (
                    tc.tile_pool(name="g1", bufs=2) as gpool,
                    tc.tile_pool(name="oh1", bufs=32) as ohpool,
                    tc.tile_pool(name="ep1", bufs=2) as eppool,
                    tc.tile_pool(name="ps1", bufs=2, space="PSUM") as pspool,
                    tc.tile_pool(name="pt1", bufs=2, space="PSUM") as ptpool,
                ):
                    for w in range(cfg.NWIN):
                        tiles = [None, None]
                        for h in (0, 1):
                            cls = classes1[w][h]
                            if not cls:
                                continue
                            alloc = int(nchmax1[h]) if w < 2 \
                                else int(nch_wh[w, h])
                            t = gpool.tile([128, alloc, 2 * F1], BF16,
                                           tag=f"g1h{h}")
                            if w < 2:
                                # finite contents under every pad slot: a NaN
                                # bit pattern x 0-weight one-hot would poison
                                # the matmul accumulator
                                nc.gpsimd.memset(t[:], 0.0)
                            for (g, cb, c0, nchg, ns, mb) in cls:
                                src = ag_d[g][0:cfg.LO, :] if h == 0 \
                                    else ag_d[g][cfg.LO:cfg.NPAD, :]
                                nc.gpsimd.dma_gather(
                                    out_ap=t[:, c0:c0 + nchg, :],
                                    in_ap=src,
                                    idxs_ap=idx1s[:, cb:cb + ns // 16],
                                    num_idxs=ns,
                                    num_idxs_reg=ns,
                                    elem_size=2 * F1,
                                    single_packet=False,
                                )
                            tiles[h] = t
                        acc = pspool.tile([F1, WIN], F32, tag="acc")
                        nmm = 2 * int(nch_wh[w, 0] + nch_wh[w, 1])
                        k = 0
                        for h in (0, 1):
                            for (g, cb, c0, nchg, ns, mb) in classes1[w][h]:
                                for ck in range(nchg):
                                    mc = mb + ck
                                    ohA = ohpool.tile([128, WIN], BF16,
                                                      tag="oh")
                                    nc.vector.tensor_scalar(
                                        out=ohA[:], in0=io16[:],
                                        scalar1=dstA[:, mc:mc + 1],
                                        scalar2=wgtA[:, mc:mc + 1],
                                        op0=mybir.AluOpType.is_equal,
                                        op1=mybir.AluOpType.mult)
                                    nc.tensor.matmul(
                                        out=acc[:],
                                        lhsT=tiles[h][:, c0 + ck, 0:F1],
                                        rhs=ohA[:],
                                        start=(k == 0), stop=(k == nmm - 1))
                                    k += 1
                                    ohB = ohpool.tile([128, WIN], BF16,
                                                      tag="oh")
                                    nc.vector.tensor_scalar(
                                        out=ohB[:], in0=io16[:],
                                        scalar1=dstB[:, mc:mc + 1],
                                        scalar2=wgtB[:, mc:mc + 1],
                                        op0=mybir.AluOpType.is_equal,
                                        op1=mybir.AluOpType.mult)
                                    nc.tensor.matmul(
                                        out=acc[:],
                                        lhsT=tiles[h][:, c0 + ck, F1:2 * F1],
                                        rhs=ohB[:],
                                        start=(k == 0), stop=(k == nmm - 1))
                                    k += 1
                        # window epilogue
                        mbf = eppool.tile([F1, WIN], BF16, tag="mbf")
                        nc.scalar.activation(
                            out=mbf[:], in_=acc[:],
                            func=mybir.ActivationFunctionType.Identity)
                        z = ptpool.tile([F2, WIN], F32, tag="z")
                        nc.tensor.matmul(out=z[:], lhsT=w1[:],
                                         rhs=mbf[:], start=True,
                                         stop=True)
                        ht = eppool.tile([F2, WIN], BF16, tag="ht")
                        nc.scalar.activation(
                            out=ht[:], in_=z[:],
                            func=mybir.ActivationFunctionType.Relu,
                            bias=b1[:, 0:1])
                        for hf in range(WIN // 128):
                            hh = w * (WIN // 128) + hf
                            if hh >= HALVES:
                                continue
                            tp = ptpool.tile([128, 128], BF16, tag="tp")
                            nc.tensor.transpose(
                                out=tp[:],
                                in_=ht[:, hf * 128:(hf + 1) * 128],
                                identity=id16[:])
                            nc.scalar.activation(
                                out=h_sb[:, hh, :], in_=tp[:],
                                func=mybir.ActivationFunctionType.Identity)
                        # ship this window's h rows to DRAM now so the
                        # allgather input is ready as soon as L1 ends
                        hh0 = w * (WIN // 128)
                        hh1 = min(hh0 + WIN // 128, HALVES)
                        if hh1 > hh0:
                            nc.sync.dma_start(
                                out=h_shard[:].rearrange(
                                    "(hh p) f -> p hh f", p=128)[:, hh0:hh1, :],
                                in_=h_sb[:, hh0:hh1, :])
                        # fire quarter AllGathers mid-stream, LAG windows
                        # after the quarter's last window was issued, so
                        # the Pool engine never stalls waiting for compute
                        for q in range(cfg.QN):
                            if w + 1 == min(cfg.QWIN[q + 1] + 2,
                                            cfg.NWIN) and \
                                    cfg.QWIN[q + 1] + 2 <= cfg.NWIN:
                                emit_ag(q)

            def run_layer(layer):
                assert layer == 2
                elem, gdt = F2, BF16
                src_lo = h_full[0:cfg.LO, :]
                src_hi = h_full[cfg.LO:cfg.NPAD, :]
                with (
                    tc.tile_pool(name=f"g{layer}", bufs=2) as gpool,
                    tc.tile_pool(name=f"oh{layer}", bufs=32) as ohpool,
                    tc.tile_pool(name=f"ep{layer}", bufs=2) as eppool,
                    tc.tile_pool(name=f"ps{layer}", bufs=2, space="PSUM") as pspool,
                    tc.tile_pool(name=f"pt{layer}", bufs=2, space="PSUM") as ptpool,
                ):
                    piece_of = {}
                    g_lo_t = {}
                    g_hi_t = {}
                    for pi, (w0, w1_) in enumerate(pieces):
                        for w in range(w0, w1_):
                            piece_of[w] = pi

                    def need_piece(pi):
                        if pi in g_lo_t:
                            return
                        w0, w1_ = pieces[pi]
                        ncl = int(LB[w1_] - LB[w0])
                        nch = int(HB[w1_] - HB[w0])
                        g_lo = gpool.tile([128, ncl, elem], gdt, tag="glo")
                        g_hi = gpool.tile([128, nch, elem], gdt, tag="ghi")
                        gather(g_lo[:, :, :], src_lo, int(LB[w0]), ncl, elem)
                        gather(g_hi[:, :, :], src_hi, int(tot_lo + HB[w0]),
                               nch, elem)
                        g_lo_t[pi] = g_lo
                        g_hi_t[pi] = g_hi

                    def glo_chunk(w, gc):
                        pi = piece_of[w]
                        need_piece(pi)
                        return g_lo_t[pi][:, gc - int(LB[pieces[pi][0]]), :]

                    def ghi_chunk(w, gc):
                        pi = piece_of[w]
                        need_piece(pi)
                        return g_hi_t[pi][:, gc - int(HB[pieces[pi][0]]), :]

                    for w in range(cfg.NWIN):
                        nl, nh = int(ch_lo[w]), int(ch_hi[w])
                        acc = pspool.tile([F2, WIN], F32, tag="acc")
                        tot = nl + nh
                        for k in range(tot):
                            if k < nl:
                                g = glo_chunk(w, int(LB[w]) + k)
                            else:
                                g = ghi_chunk(w, int(HB[w]) + k - nl)
                            mc = int(MB[w]) + k
                            oh = ohpool.tile([128, WIN], BF16, tag="oh")
                            nc.vector.tensor_scalar(
                                out=oh[:], in0=io16[:],
                                scalar1=dstrow[:, mc:mc + 1],
                                scalar2=wgt[:, mc:mc + 1],
                                op0=mybir.AluOpType.is_equal,
                                op1=mybir.AluOpType.mult)
                            nc.tensor.matmul(
                                out=acc[:], lhsT=g, rhs=oh[:],
                                start=(k == 0), stop=(k == tot - 1))
                        # window epilogue
                        if True:
                            mbf = eppool.tile([F2, WIN], BF16, tag="mbf")
                            nc.scalar.activation(
                                out=mbf[:], in_=acc[:],
                                func=mybir.ActivationFunctionType.Identity)
                            z = ptpool.tile([F3, WIN], F32, tag="z")
                            nc.tensor.matmul(out=z[:], lhsT=w2[:],
                                             rhs=mbf[:], start=True,
                                             stop=True)
                            o2 = eppool.tile([128, WIN], F32, tag="ht")
                            nc.scalar.activation(
                                out=o2[0:F3, :], in_=z[:],
                                func=mybir.ActivationFunctionType.Identity,
                                bias=b2[:, 0:1])
                            for hf in range(WIN // 128):
                                hh = w * (WIN // 128) + hf
                                if hh >= HALVES:
                                    continue
                                tp = ptpool.tile([128, 128], F32, tag="tp")
                                nc.tensor.transpose(
                                    out=tp[:],
                                    in_=o2[:, hf * 128:(hf + 1) * 128],
                                    identity=id32[:])
                                nc.scalar.activation(
                                    out=out_sb[:, hh, :], in_=tp[:, 0:F3],
                                    func=mybir.ActivationFunctionType.Identity)
                            # quarter boundary: log_softmax + ship output rows
                            for q in range(cfg.QN):
                                q0 = int(cfg.QROWS[q]) // 128
                                q1 = int(cfg.QROWS[q + 1]) // 128
                                if w == (q1 - 1) // (WIN // 128):
                                    nh = q1 - q0
                                    nhx = (int(cfg.QROWS[-1]) -
                                           int(cfg.QROWS[-2])) // 128
                                    nhx = max(nhx, nh)
                                    sl = out_sb[:, q0:q1, :]
                                    mx = eppool.tile([128, nhx, 1], F32,
                                                     tag="smx")
                                    nc.vector.reduce_max(
                                        out=mx[:, 0:nh, :], in_=sl,
                                        axis=mybir.AxisListType.X)
                                    nc.vector.tensor_tensor(
                                        out=sl, in0=sl,
                                        in1=mx[:, 0:nh, :].to_broadcast(
                                            [128, nh, F3]),
                                        op=mybir.AluOpType.subtract)
                                    ex = eppool.tile([128, nhx, F3], F32,
                                                     tag="sex")
                                    nc.scalar.activation(
                                        out=ex[:, 0:nh, :], in_=sl,
                                        func=mybir.ActivationFunctionType.Exp)
                                    sm = eppool.tile([128, nhx, 1], F32,
                                                     tag="ssm")
                                    nc.vector.reduce_sum(
                                        out=sm[:, 0:nh, :], in_=ex[:, 0:nh, :],
                                        axis=mybir.AxisListType.X)
                                    ls = eppool.tile([128, nhx, 1], F32,
                                                     tag="sls")
                                    nc.scalar.activation(
                                        out=ls[:, 0:nh, :], in_=sm[:, 0:nh, :],
                                        func=mybir.ActivationFunctionType.Ln)
                                    nc.vector.tensor_tensor(
                                        out=sl, in0=sl,
                                        in1=ls[:, 0:nh, :].to_broadcast(
                                            [128, nh, F3]),
                                        op=mybir.AluOpType.subtract)
                                    nc.sync.dma_start(
                                        out=out_d[:, q0:q1, :], in_=sl)

            import os
            stage = os.environ.get("K_STAGE", "full")
            run_layer1()
            if stage != "l1":
                # quarters whose (end + LAG) passed NWIN fire here, after the
                # last L1 window was issued
                for q in range(cfg.QN):
                    if cfg.QWIN[q + 1] + 2 > cfg.NWIN:
                        emit_ag(q)
            if stage == "full":
                run_layer(2)

    nc.compile()
    return nc


_CACHE = {}


def _get_program(cfg, x, src, dst, W1, b1, W2, b2):
    deg = np.bincount(dst, minlength=cfg.N).astype(np.float64)
    deg_w = (1.0 / (deg + 1e-6)).astype(np.float32)
    struct1, per_core1 = prep1(cfg, src, dst, deg_w)
    struct, per_core = prep(cfg, src, dst, deg_w)

    # bf16 x in padded-id order, plus one pair-layout per gap value g:
    # ag[s] = [x[s] | x[s+g]] so one 256B descriptor serves two edges
    xp = np.zeros((cfg.NPAD + cfg.G + 1, cfg.F1), ml_dtypes.bfloat16)
    xp[pid_map(cfg, np.arange(cfg.N))] = x.astype(ml_dtypes.bfloat16)
    ags = {}
    for g in range(cfg.G + 1):
        ags[f"ag{g}"] = np.concatenate(
            [xp[:cfg.NPAD], xp[g:cfg.NPAD + g]], axis=1)

    iota = np.arange(cfg.WIN, dtype=np.float32)
    shared = dict(
        W1b=W1.astype(ml_dtypes.bfloat16),
        W2b=W2.astype(ml_dtypes.bfloat16),
        b1d=b1.reshape(-1, 1).astype(np.float32),
        b2d=b2.reshape(-1, 1).astype(np.float32),
        iota16=np.tile(iota, (128, 1)).astype(np.float16),
        ident16=np.eye(128, dtype=ml_dtypes.bfloat16),
        ident32=np.eye(128, dtype=np.float32),
        **ags,
    )
    in_maps = []
    for c in range(cfg.NC):
        m = dict(shared)
        m["idx"] = per_core[c]["idx"]
        m["dstrow"] = per_core[c]["dstrow"]
        m["wgt"] = per_core[c]["wgt"]
        m["idx1"] = per_core1[c]["idx1"]
        m["dstA"] = per_core1[c]["dstA"]
        m["wgtA"] = per_core1[c]["wgtA"]
        m["dstB"] = per_core1[c]["dstB"]
        m["wgtB"] = per_core1[c]["wgtB"]
        in_maps.append(m)

    key = (cfg.N, cfg.E, struct["totch"], tuple(struct["ch_lo"]),
           tuple(struct["ch_hi"]), struct1["totch1"], struct1["icols1"],
           tuple(tuple(tuple(cl) for cl in classes_h)
                 for ww in struct1["classes"] for classes_h in ww))
    if key not in _CACHE:
        _CACHE[key] = build_program(cfg, struct1, struct)
    return _CACHE[key], in_maps


def run(cfg, x, src, dst, W1, b1, W2, b2, trace=False, trace_kwargs=None):
    nc, in_maps = _get_program(cfg, x, src, dst, W1, b1, W2, b2)
    res = run_bass_kernel_spmd(nc, in_maps, core_ids=list(range(cfg.NC)),
                               trace=trace, **(trace_kwargs or {}))
    out = np.empty((cfg.N, cfg.F3), np.float32)
    for c in range(cfg.NC):
        o = np.asarray(res.results[c]["out"])  # [128, HALVES, F3]
        o = o.transpose(1, 0, 2).reshape(cfg.RPAD, cfg.F3)
        out[c * cfg.RPC:(c + 1) * cfg.RPC] = o[:cfg.RPC]
    return out, res


def kernel(x, src, dst, W1, b1, W2, b2):
    cfg = Cfg()
    out, _ = run(cfg, np.asarray(x, np.float32), np.asarray(src),
                 np.asarray(dst), np.asarray(W1, np.float32),
                 np.asarray(b1, np.float32), np.asarray(W2, np.float32),
                 np.asarray(b2, np.float32))
    return out

